# revision 17
# baseline (speedup 1.0000x reference)
"""Trainium2 Bass kernel for nn_GammaCapsGraph (capsule routing over gram matrix).

Math (per batch, X = x[b] of shape (D=128, N=1024)):
  G = X^T X (symmetric gram), u_norm = sqrt(diag G), u_hat_norm = ||G row||
  U = alpha * G rowwise, alpha = min(u_hat_norm, u_norm)/u_hat_norm
  3 routing iterations where c is a per-row scalar, so all row reductions
  collapse onto row stats:
    q[n] = min(u_hat_norm,u_norm)^2, rr[n] = alpha*bias_n*rowsum(G),
    bb[n] = N*bias_n^2       (bias verified row-constant on host)
    sq = c^2 q + 2c rr + bb;  f = sqrt(sq)/(1+sq)
    d^2 = f^2 sq + (1-2fc) q - 2f rr;  d_o = global mean(d) -> t -> c' = softmax(t d)
  Output v = (f*c*alpha) * G + (f*bias_n).

Structure (per core, 4 local batches):
  1. Gram chunks (b,ch) -> PSUM; fused ACT copy(+rowsum accum) stages G in
     SBUF; DVE tensor_tensor_reduce gives ssq (row norms^2); masked reduce
     gives diag. Stats come out in column layout packs (128, 32).
  2. Derived q/rr packs -> PE transpose -> (32, 256) stage -> ONE AllGather
     (vs two AllReduces in the serial formulation): every core gets all 32
     batches' (q, rr) and runs the routing redundantly -> t0, t1, c2, f2
     locally with no further communication.
  3. Per-core (f2*c2, f2) extracted via selection matmuls, combined with
     alpha -> per-row scale/bias; in-place scale on staged G; stream out.
A dummy AllReduce is issued first thing to pre-pay the collectives
rendezvous barrier concurrently with the gram phase.
"""
import os

import numpy as np

import concourse.bass as bass
import concourse.bacc as bacc
import concourse.tile as tile
import concourse.mybir as mybir
from concourse.bass_utils import run_bass_kernel_spmd

N_CORES = 8
B_LOC = 4
D = 128
N = 1024
NCH = 8  # column chunks of 128
NC32 = B_LOC * NCH  # 32 (b, ch) pairs
P_P = 0.9
NUM_SECONDARY = 1024
EPS = 1e-12
T_NUM = float(np.log(P_P * (NUM_SECONDARY - 1)) - np.log(1.0 - P_P))
C0 = 1.0 / N

F = mybir.dt.float32
FR = mybir.dt.float32r
AF = mybir.ActivationFunctionType
OP = mybir.AluOpType
AX = mybir.AxisListType

LAST_EXEC_NS = None
_NC_CACHE = None


def _build():
    nc = bacc.Bacc("TRN2", target_bir_lowering=False, debug=False,
                   enable_asserts=False, num_devices=N_CORES)
    xs = nc.dram_tensor("xs", (B_LOC, D, N), FR, kind="ExternalInput").ap()
    iden_in = nc.dram_tensor("iden", (D, D), F, kind="ExternalInput").ap()
    m16_in = nc.dram_tensor("m16", (D, 16), F, kind="ExternalInput").ap()
    m16t_in = nc.dram_tensor("m16t", (16, D), F, kind="ExternalInput").ap()
    sel0_in = nc.dram_tensor("sel0", (D, NC32), F, kind="ExternalInput").ap()
    sel1_in = nc.dram_tensor("sel1", (D, NC32), F, kind="ExternalInput").ap()
    sel0t_in = nc.dram_tensor("sel0t", (NC32, D), F, kind="ExternalInput").ap()
    sel1t_in = nc.dram_tensor("sel1t", (NC32, D), F, kind="ExternalInput").ap()
    bcol_in = nc.dram_tensor("bcol4", (D, NC32), F, kind="ExternalInput").ap()
    bb2_in = nc.dram_tensor("bb2", (D, 256), F, kind="ExternalInput").ap()
    vout = nc.dram_tensor("v", (B_LOC, N, N), F, kind="ExternalOutput").ap()

    rg = [list(range(N_CORES))]

    with tile.TileContext(nc) as tc:
        with (
            tc.tile_pool(name="const", bufs=1) as cpool,
            tc.tile_pool(name="xp", bufs=1) as xp,
            tc.tile_pool(name="gsb", bufs=1) as gsb,
            tc.tile_pool(name="scr", bufs=1) as scr,
            tc.tile_pool(name="pk", bufs=1) as pk,
            tc.tile_pool(name="row", bufs=1) as row,
            tc.tile_pool(name="psb", bufs=3, space="PSUM") as psb,
            tc.tile_pool(name="pss", bufs=2, space="PSUM") as pss,
            tc.tile_pool(name="dram", bufs=1, space="DRAM") as dram,
        ):
            _cnt = [0]

            def _nm(tag):
                _cnt[0] += 1
                return f"{tag}_{_cnt[0]}"

            # ---- constants ----
            ident = cpool.tile([D, D], F)
            nc.scalar.dma_start(ident[:], iden_in[:])
            m16 = cpool.tile([D, 16], F)
            nc.scalar.dma_start(m16[:], m16_in[:])
            m16t = cpool.tile([16, D], F)
            nc.scalar.dma_start(m16t[:], m16t_in[:])
            sel0 = cpool.tile([D, NC32], F)
            nc.scalar.dma_start(sel0[:], sel0_in[:])
            sel1 = cpool.tile([D, NC32], F)
            nc.scalar.dma_start(sel1[:], sel1_in[:])
            sel0t = cpool.tile([NC32, D], F)
            nc.scalar.dma_start(sel0t[:], sel0t_in[:])
            sel1t = cpool.tile([NC32, D], F)
            nc.scalar.dma_start(sel1t[:], sel1t_in[:])
            bcol4 = cpool.tile([D, NC32], F)
            nc.scalar.dma_start(bcol4[:], bcol_in[:])
            bb2 = cpool.tile([D, 256], F)
            nc.scalar.dma_start(bb2[:], bb2_in[:])
            ones128 = cpool.tile([D, 1], F)
            nc.vector.memset(ones128[:], 1.0)
            onesr = cpool.tile([1, D], F)
            nc.vector.memset(onesr[:], 1.0)

            # ---- load x directly as fp32r (same bits as fp32) ----
            xfr = [xp.tile([D, N], FR, tag=f"fx{b}", name=f"fx{b}")
                   for b in range(B_LOC)]
            for b in range(B_LOC):
                nc.sync.dma_start(xfr[b][:], xs[b])

            # ---- stat packs, column layout [p, 8b+ch] ----
            diag_pk = pk.tile([D, NC32], F)
            rsum_pk = pk.tile([D, NC32], F)
            ssq_pk = pk.tile([D, NC32], F)

            # ---- gram chunks + fused stats ----
            gt = [gsb.tile([D, N], F, tag=f"g{i}", name=f"g{i}")
                  for i in range(NC32)]
            for i in range(NC32):
                b, ch = divmod(i, NCH)
                gps = psb.tile([D, N], F, tag="big")
                lhs = xfr[b][:, 128 * ch:128 * (ch + 1)]
                nc.tensor.matmul(gps[:, 0:512], lhs, xfr[b][:, 0:512],
                                 start=True, stop=True)
                nc.tensor.matmul(gps[:, 512:1024], lhs, xfr[b][:, 512:1024],
                                 start=True, stop=True)
                # stage to SBUF + rowsum(G) in one ACT pass
                nc.scalar.activation(gt[i][:], gps[:], AF.Identity,
                                     accum_out=rsum_pk[:, i:i + 1])
                # ssq = rowsum(G*G): square on GpSimd (Pool), reduce on DVE
                sq_scr = scr.tile([D, N], F, tag="sqscr", name=_nm("sqscr"))
                nc.gpsimd.tensor_tensor(sq_scr[:], gt[i][:], gt[i][:],
                                        op=OP.mult)
                nc.vector.reduce_sum(ssq_pk[:, i:i + 1], sq_scr[:], axis=AX.X)
                # diag = rowsum(G_block * I)
                dg_scr = scr.tile([D, D], F, tag="dgscr", name=_nm("dgscr"))
                nc.vector.tensor_tensor(dg_scr[:],
                                        gps[:, 128 * ch:128 * (ch + 1)],
                                        ident[:], op=OP.mult)
                nc.vector.reduce_sum(diag_pk[:, i:i + 1], dg_scr[:],
                                     axis=AX.X)

            # ---- derived stats (column layout) ----
            un = pk.tile([D, NC32], F)
            nc.scalar.activation(un[:], diag_pk[:], AF.Sqrt)
            uh = pk.tile([D, NC32], F)
            nc.scalar.activation(uh[:], ssq_pk[:], AF.Sqrt)
            nn_t = pk.tile([D, NC32], F)
            nc.vector.tensor_tensor(nn_t[:], uh[:], un[:], op=OP.min)
            q_pack = pk.tile([D, NC32], F)
            nc.vector.tensor_tensor(q_pack[:], nn_t[:], nn_t[:], op=OP.mult)
            ivh = pk.tile([D, NC32], F)
            nc.vector.reciprocal(ivh[:], uh[:])
            alpha = pk.tile([D, NC32], F)
            nc.vector.tensor_tensor(alpha[:], nn_t[:], ivh[:], op=OP.mult)
            rr_t = pk.tile([D, NC32], F)
            nc.vector.tensor_tensor(rr_t[:], alpha[:], rsum_pk[:], op=OP.mult)
            rr_pack = pk.tile([D, NC32], F)
            nc.vector.tensor_tensor(rr_pack[:], rr_t[:], bcol4[:], op=OP.mult)

            # ---- transpose packs to row layout, stage, AllGather ----
            stage = pk.tile([NC32, 256], F)
            qs_ps = pss.tile([NC32, D], F, tag="small", name=_nm("ps_qs"))
            nc.tensor.transpose(qs_ps[:], q_pack[:], ident[:])
            nc.scalar.copy(stage[:, 0:128], qs_ps[:])
            rs_ps = pss.tile([NC32, D], F, tag="small", name=_nm("ps_rs"))
            nc.tensor.transpose(rs_ps[:], rr_pack[:], ident[:])
            nc.scalar.copy(stage[:, 128:256], rs_ps[:])

            # gather emulated as a masked AllReduce: scatter our 32 stat rows
            # into the right global rows (zeros elsewhere) via selection
            # matmuls, then sum across cores.
            full0_ps = pss.tile([D, 256], F, tag="small", name=_nm("ps_f0"))
            nc.tensor.matmul(full0_ps[:], sel0t[:], stage[:],
                             start=True, stop=True)
            full0 = row.tile([D, 256], F, tag="full0", name="full0")
            nc.scalar.copy(full0[:], full0_ps[:])
            full1_ps = pss.tile([D, 256], F, tag="small", name=_nm("ps_f1"))
            nc.tensor.matmul(full1_ps[:], sel1t[:], stage[:],
                             start=True, stop=True)
            full1 = row.tile([D, 256], F, tag="full1", name="full1")
            nc.scalar.copy(full1[:], full1_ps[:])

            ag_in = dram.tile([N_CORES * NC32, 256], F, tag="agin")
            ag_out = dram.tile([N_CORES * NC32, 256], F, tag="agout",
                               addr_space="Shared")
            nc.sync.dma_start(ag_in[0:128, :], full0[:])
            nc.scalar.dma_start(ag_in[128:256, :], full1[:])
            nc.gpsimd.collective_compute(
                "AllReduce", OP.add, replica_groups=rg,
                ins=[ag_in.opt()], outs=[ag_out.opt()])

            # TT layout: [q(T0) | q(T1) | rr(T0) | rr(T1)], each (128,128)
            TT = row.tile([D, 512], F, tag="TT", name="TT")
            nc.sync.dma_start(TT[:, 0:128], ag_out[0:128, 0:128])
            nc.scalar.dma_start(TT[:, 128:256], ag_out[128:256, 0:128])
            nc.sync.dma_start(TT[:, 256:384], ag_out[0:128, 128:256])
            nc.scalar.dma_start(TT[:, 384:512], ag_out[128:256, 128:256])
            qm = TT[:, 0:256]
            rm = TT[:, 256:512]

            # ---- routing (redundant, all 32 batches) ----
            def row_t(tag):
                return row.tile([D, 256], F, tag=tag, name=_nm(tag))

            def t_chain(d):
                """global scalar t from d tile -> broadcast (128,1)"""
                part = row.tile([D, 1], F, tag="part", name=_nm("part"))
                nc.vector.reduce_sum(part[:], d[:], axis=AX.X)
                tot_ps = pss.tile([1, 1], F, tag="small", name=_nm("ps_tot"))
                nc.tensor.matmul(tot_ps[:], part[:], ones128[:],
                                 start=True, stop=True)
                tot = row.tile([1, 1], F, tag="tot", name=_nm("tot"))
                nc.scalar.copy(tot[:], tot_ps[:])
                dent = row.tile([1, 1], F, tag="dent", name=_nm("dent"))
                nc.vector.tensor_scalar(dent[:], tot[:], -0.5 / 32768.0, EPS,
                                        op0=OP.mult, op1=OP.add)
                it = row.tile([1, 1], F, tag="it", name=_nm("it"))
                nc.vector.reciprocal(it[:], dent[:])
                tv = row.tile([1, 1], F, tag="tv", name=_nm("tv"))
                nc.vector.tensor_scalar_mul(tv[:], it[:], T_NUM)
                tb_ps = pss.tile([D, 1], F, tag="small", name=_nm("ps_tb"))
                nc.tensor.matmul(tb_ps[:], onesr[:], tv[:],
                                 start=True, stop=True)
                tb = row.tile([D, 1], F, tag="tb", name=_nm("tb"))
                nc.scalar.copy(tb[:], tb_ps[:])
                return tb

            def softmax_c(d, tb):
                e = row_t("e")
                nc.scalar.activation(e[:], d[:], AF.Exp, scale=tb[:])
                pe2 = row.tile([D, 2], F, tag="pe2", name=_nm("pe2"))
                nc.vector.reduce_sum(pe2[:, 0:1], e[:, 0:128], axis=AX.X)
                nc.vector.reduce_sum(pe2[:, 1:2], e[:, 128:256], axis=AX.X)
                bs_ps = pss.tile([16, 2], F, tag="small", name=_nm("ps_bs"))
                nc.tensor.matmul(bs_ps[:], m16[:], pe2[:],
                                 start=True, stop=True)
                bs = row.tile([16, 2], F, tag="bs", name=_nm("bs"))
                nc.scalar.copy(bs[:], bs_ps[:])
                binv = row.tile([16, 2], F, tag="binv", name=_nm("binv"))
                nc.vector.reciprocal(binv[:], bs[:])
                ib_ps = pss.tile([D, 2], F, tag="small", name=_nm("ps_ib"))
                nc.tensor.matmul(ib_ps[:], m16t[:], binv[:],
                                 start=True, stop=True)
                ib = row.tile([D, 2], F, tag="ib", name=_nm("ib"))
                nc.scalar.copy(ib[:], ib_ps[:])
                c = row_t("c")
                nc.vector.tensor_scalar(c[:, 0:128], e[:, 0:128],
                                        ib[:, 0:1], None, op0=OP.mult)
                nc.vector.tensor_scalar(c[:, 128:256], e[:, 128:256],
                                        ib[:, 1:2], None, op0=OP.mult)
                return c

            def compute_sq_f(c):
                """sq = c*(c*q + 2rr) + bb; f = sqrt(sq)/(1+sq)"""
                u = row_t("u")
                nc.vector.tensor_tensor(u[:], c[:], qm, op=OP.mult)
                nc.vector.scalar_tensor_tensor(u[:], rm, 2.0, u[:],
                                               op0=OP.mult, op1=OP.add)
                sq = row_t("sq")
                nc.vector.tensor_tensor(sq[:], c[:], u[:], op=OP.mult)
                nc.vector.tensor_tensor(sq[:], sq[:], bb2[:], op=OP.add)
                return sq, _f_of(sq)

            def _f_of(sq):
                sqs = row_t("sqs")
                nc.scalar.activation(sqs[:], sq[:], AF.Sqrt)
                den = row_t("den")
                nc.vector.tensor_scalar_add(den[:], sq[:], 1.0)
                inv = row_t("inv")
                nc.vector.reciprocal(inv[:], den[:])
                f = row_t("f")
                nc.vector.tensor_tensor(f[:], sqs[:], inv[:], op=OP.mult)
                return f

            def compute_d(f, sq, w):
                """d = sqrt(f^2 sq + w*q - 2 f rr), w = 1-2fc precomputed"""
                a1 = row_t("a1")
                nc.vector.tensor_tensor(a1[:], f[:], sq[:], op=OP.mult)
                nc.vector.tensor_tensor(a1[:], a1[:], f[:], op=OP.mult)
                a3 = row_t("a3")
                nc.vector.tensor_tensor(a3[:], w[:], qm, op=OP.mult)
                a4 = row_t("a4")
                nc.vector.tensor_tensor(a4[:], f[:], rm, op=OP.mult)
                d2 = row_t("d2")
                nc.vector.scalar_tensor_tensor(d2[:], a4[:], -2.0, a1[:],
                                               op0=OP.mult, op1=OP.add)
                nc.vector.tensor_tensor(d2[:], d2[:], a3[:], op=OP.add)
                d = row_t("d")
                nc.scalar.activation(d[:], d2[:], AF.Sqrt)
                return d

            # iteration 0: c = C0 scalar
            sq0 = row_t("sq0")
            nc.vector.scalar_tensor_tensor(sq0[:], rm, 2.0 * C0, bb2[:],
                                           op0=OP.mult, op1=OP.add)
            nc.vector.scalar_tensor_tensor(sq0[:], qm, C0 * C0, sq0[:],
                                           op0=OP.mult, op1=OP.add)
            f0 = _f_of(sq0)
            w0 = row_t("w0")
            nc.vector.tensor_scalar(w0[:], f0[:], -2.0 * C0, 1.0,
                                    op0=OP.mult, op1=OP.add)
            d0 = compute_d(f0, sq0, w0)
            tb0 = t_chain(d0)
            # iteration 1
            c1 = softmax_c(d0, tb0)
            sq1, f1 = compute_sq_f(c1)
            w1 = row_t("w1")
            fc1 = row_t("fc1")
            nc.vector.tensor_tensor(fc1[:], f1[:], c1[:], op=OP.mult)
            nc.vector.tensor_scalar(w1[:], fc1[:], -2.0, 1.0,
                                    op0=OP.mult, op1=OP.add)
            d1 = compute_d(f1, sq1, w1)
            tb1 = t_chain(d1)
            # iteration 2 (final): only c2, f2 needed
            c2 = softmax_c(d1, tb1)
            _, f2 = compute_sq_f(c2)
            fc2 = row_t("fc2")
            nc.vector.tensor_tensor(fc2[:], f2[:], c2[:], op=OP.mult)

            # ---- extract our 4 batches to column layout via selection ----
            fcT_ps = pss.tile([D, NC32], F, tag="small", name=_nm("ps_fcT"))
            nc.tensor.matmul(fcT_ps[:], fc2[:, 0:128], sel0[:],
                             start=True, stop=False)
            nc.tensor.matmul(fcT_ps[:], fc2[:, 128:256], sel1[:],
                             start=False, stop=True)
            fcT = pk.tile([D, NC32], F)
            nc.scalar.copy(fcT[:], fcT_ps[:])
            fT_ps = pss.tile([D, NC32], F, tag="small", name=_nm("ps_fT"))
            nc.tensor.matmul(fT_ps[:], f2[:, 0:128], sel0[:],
                             start=True, stop=False)
            nc.tensor.matmul(fT_ps[:], f2[:, 128:256], sel1[:],
                             start=False, stop=True)
            fT = pk.tile([D, NC32], F)
            nc.scalar.copy(fT[:], fT_ps[:])

            a_col = pk.tile([D, NC32], F)
            nc.vector.tensor_tensor(a_col[:], fcT[:], alpha[:], op=OP.mult)
            c_col = pk.tile([D, NC32], F)
            nc.vector.tensor_tensor(c_col[:], fT[:], bcol4[:], op=OP.mult)

            # ---- output: in-place scale on staged G, stream out ----
            for i in range(NC32):
                b, ch = divmod(i, NCH)
                r = i % 8
                if r < 3:
                    nc.vector.tensor_scalar(gt[i][:], gt[i][:],
                                            a_col[:, i:i + 1],
                                            c_col[:, i:i + 1],
                                            op0=OP.mult, op1=OP.add)
                elif r < 6:
                    nc.gpsimd.tensor_scalar(gt[i][:], gt[i][:],
                                            a_col[:, i:i + 1],
                                            c_col[:, i:i + 1],
                                            op0=OP.mult, op1=OP.add)
                else:
                    nc.scalar.activation(gt[i][:], gt[i][:], AF.Identity,
                                         bias=c_col[:, i:i + 1],
                                         scale=a_col[:, i:i + 1])
                dma_eng = nc.sync if i % 2 == 0 else nc.scalar
                dma_eng.dma_start(vout[b, 128 * ch:128 * (ch + 1), :],
                                  gt[i][:])

    nc.compile()
    return nc


def _get_nc():
    global _NC_CACHE
    if _NC_CACHE is None:
        _NC_CACHE = _build()
    return _NC_CACHE


def _make_host_inputs():
    iden = np.eye(D, dtype=np.float32)
    m16 = np.zeros((D, 16), dtype=np.float32)
    m16t = np.zeros((16, D), dtype=np.float32)
    for g in range(16):
        m16[8 * g:8 * g + 8, g] = 1.0
        m16t[g, 8 * g:8 * g + 8] = 1.0
    return iden, m16, m16t


def _make_sel(core):
    sel0 = np.zeros((D, NC32), dtype=np.float32)
    sel1 = np.zeros((D, NC32), dtype=np.float32)
    for b in range(B_LOC):
        g = 4 * core + b
        for c in range(NCH):
            if g < 16:
                sel0[8 * g + c, 8 * b + c] = 1.0
            else:
                sel1[8 * (g - 16) + c, 8 * b + c] = 1.0
    return sel0, sel1


def _reference_numpy(x, bias):
    """General fallback (non-row-constant bias): straight numpy port."""
    x = x.astype(np.float32)
    bias = bias.astype(np.float32)
    u_norm = np.linalg.norm(x, axis=1)[..., None]
    u_hat = np.einsum('bdn,bdm->bnm', x, x)
    u_hat_norm = np.linalg.norm(u_hat, axis=-1, keepdims=True)
    new_norm = np.minimum(u_hat_norm, u_norm)
    u_hat = u_hat / u_hat_norm * new_norm
    t_num = np.float32(T_NUM)
    b_ij = np.zeros(u_hat.shape, dtype=np.float32)
    v_j = None
    for it in range(3):
        m = b_ij.max(axis=1, keepdims=True)
        e = np.exp(b_ij - m)
        c_ij = e / e.sum(axis=1, keepdims=True)
        s_j = c_ij * u_hat + bias
        sqn = np.sum(s_j * s_j, axis=-1, keepdims=True)
        v_j = sqn * s_j / ((1.0 + sqn) * np.sqrt(sqn))
        if it < 2:
            dd = np.linalg.norm(v_j - u_hat, axis=-1, keepdims=True)
            d_o = dd.mean()
            t = t_num / (0.5 * d_o - d_o + EPS)
            b_ij = t * dd
    return v_j


def kernel(x, bias):
    global LAST_EXEC_NS
    x = np.ascontiguousarray(x, dtype=np.float32)
    bias = np.ascontiguousarray(bias, dtype=np.float32)
    B = x.shape[0]
    row_const = bool((bias == bias[:, :, :1]).all())
    if not row_const or B != 32 or x.shape[1:] != (D, N):
        return _reference_numpy(x, bias)
    brow = np.ascontiguousarray(bias[0, :, 0]).astype(np.float32)  # (N,)
    iden, m16, m16t = _make_host_inputs()
    # bcol4[p, 8b+c] = bias[128c+p]; bb2[p', 128h+p] pattern rows repeat per 8
    bcol = brow.reshape(NCH, D).T  # (128, 8): [p, c]
    bcol4 = np.ascontiguousarray(np.tile(bcol, (1, B_LOC)))  # [p, 8b+c]? no:
    # tile gives [p, c*4] order [c0..c7,c0..c7,..]; need [8b+c] = same pattern
    # since tile repeats the 8-col block 4 times -> col index 8b+c maps to c ✓
    bb_row = (32.0 * brow) ** 2  # N*bias^2, (N,)
    bbp = bb_row.reshape(NCH, D)  # [c, p]
    bb128 = np.zeros((D, D), dtype=np.float32)  # [8g+c, p] rows repeat per 8
    for g in range(16):
        bb128[8 * g:8 * g + 8, :] = bbp
    bb2 = np.ascontiguousarray(np.concatenate([bb128, bb128], axis=1))
    nc = _get_nc()
    in_maps = []
    for core in range(N_CORES):
        sel0, sel1 = _make_sel(core)
        in_maps.append({
            "xs": np.ascontiguousarray(x[4 * core:4 * core + 4]),
            "iden": iden, "m16": m16, "m16t": m16t,
            "sel0": sel0, "sel1": sel1,
            "sel0t": np.ascontiguousarray(sel0.T),
            "sel1t": np.ascontiguousarray(sel1.T),
            "bcol4": bcol4, "bb2": bb2,
        })
    res = run_bass_kernel_spmd(nc, in_maps, core_ids=list(range(N_CORES)))
    LAST_EXEC_NS = res.exec_time_ns
    globals()["LAST_RES"] = res
    return np.concatenate([res.results[c]["v"] for c in range(N_CORES)], axis=0)


# revision 29
# speedup vs baseline: 1.0968x; 1.0968x over previous
"""Trainium2 Bass kernel for nn_GammaCapsGraph (capsule routing over gram matrix).

Math (per batch, X = x[b] of shape (D=128, N=1024)):
  G = X^T X (symmetric gram), u_norm = sqrt(diag G), u_hat_norm = ||G row||
  U = alpha * G rowwise, alpha = min(u_hat_norm, u_norm)/u_hat_norm
  3 routing iterations where c is a per-row scalar, so all row reductions
  collapse onto row stats:
    q[n] = min(u_hat_norm,u_norm)^2, rr[n] = alpha*bias_n*rowsum(G),
    bb[n] = N*bias_n^2       (bias verified row-constant on host)
    sq = c^2 q + 2c rr + bb;  f = sqrt(sq)/(1+sq)
    d^2 = f^2 sq + (1-2fc) q - 2f rr;  d_o = global mean(d) -> t -> c' = softmax(t d)
  Output v = (f*c*alpha) * G + (f*bias_n).

Structure (per core, 4 local batches):
  1. Gram chunks (b,ch) -> PSUM; fused ACT copy(+rowsum accum) stages G in
     SBUF; DVE tensor_tensor_reduce gives ssq (row norms^2); masked reduce
     gives diag. Stats come out in column layout packs (128, 32).
  2. Derived q/rr packs -> PE transpose -> (32, 256) stage -> ONE AllGather
     (vs two AllReduces in the serial formulation): every core gets all 32
     batches' (q, rr) and runs the routing redundantly -> t0, t1, c2, f2
     locally with no further communication.
  3. Per-core (f2*c2, f2) extracted via selection matmuls, combined with
     alpha -> per-row scale/bias; in-place scale on staged G; stream out.
A dummy AllReduce is issued first thing to pre-pay the collectives
rendezvous barrier concurrently with the gram phase.
"""
import os

import numpy as np

import concourse.bass as bass
import concourse.bacc as bacc
import concourse.tile as tile
import concourse.mybir as mybir
from concourse.bass_utils import run_bass_kernel_spmd

N_CORES = 8
B_LOC = 4
D = 128
N = 1024
NCH = 8  # column chunks of 128
NC32 = B_LOC * NCH  # 32 (b, ch) pairs
P_P = 0.9
NUM_SECONDARY = 1024
EPS = 1e-12
T_NUM = float(np.log(P_P * (NUM_SECONDARY - 1)) - np.log(1.0 - P_P))
C0 = 1.0 / N

F = mybir.dt.float32
FR = mybir.dt.float32r
BF = mybir.dt.bfloat16
AF = mybir.ActivationFunctionType
OP = mybir.AluOpType
AX = mybir.AxisListType

LAST_EXEC_NS = None
_NC_CACHE = None


def _build():
    nc = bacc.Bacc("TRN2", target_bir_lowering=False, debug=False,
                   enable_asserts=False, num_devices=N_CORES)
    xs = nc.dram_tensor("xs", (B_LOC, D, N), FR, kind="ExternalInput").ap()
    iden_in = nc.dram_tensor("iden", (D, D), F, kind="ExternalInput").ap()
    m16_in = nc.dram_tensor("m16", (D, 16), F, kind="ExternalInput").ap()
    m16t_in = nc.dram_tensor("m16t", (16, D), F, kind="ExternalInput").ap()
    sel0_in = nc.dram_tensor("sel0", (D, NC32), F, kind="ExternalInput").ap()
    sel1_in = nc.dram_tensor("sel1", (D, NC32), F, kind="ExternalInput").ap()
    bcol_in = nc.dram_tensor("bcol4", (D, NC32), F, kind="ExternalInput").ap()
    bb2_in = nc.dram_tensor("bb2", (D, 256), F, kind="ExternalInput").ap()
    vout = nc.dram_tensor("v", (B_LOC, N, N), F, kind="ExternalOutput").ap()

    rg = [list(range(N_CORES))]

    with tile.TileContext(nc) as tc:
        with (
            tc.tile_pool(name="const", bufs=1) as cpool,
            tc.tile_pool(name="xp", bufs=1) as xp,
            tc.tile_pool(name="gsb", bufs=1) as gsb,
            tc.tile_pool(name="scr", bufs=1) as scr,
            tc.tile_pool(name="pk", bufs=1) as pk,
            tc.tile_pool(name="row", bufs=1) as row,
            tc.tile_pool(name="psb", bufs=3, space="PSUM") as psb,
            tc.tile_pool(name="pss", bufs=2, space="PSUM") as pss,
            tc.tile_pool(name="dram", bufs=1, space="DRAM") as dram,
        ):
            _cnt = [0]

            def _nm(tag):
                _cnt[0] += 1
                return f"{tag}_{_cnt[0]}"

            # ---- constants ----
            ident = cpool.tile([D, D], F)
            nc.scalar.dma_start(ident[:], iden_in[:])
            m16 = cpool.tile([D, 16], F)
            nc.scalar.dma_start(m16[:], m16_in[:])
            m16t = cpool.tile([16, D], F)
            nc.scalar.dma_start(m16t[:], m16t_in[:])
            sel0 = cpool.tile([D, NC32], F)
            nc.scalar.dma_start(sel0[:], sel0_in[:])
            sel1 = cpool.tile([D, NC32], F)
            nc.scalar.dma_start(sel1[:], sel1_in[:])
            bcol4 = cpool.tile([D, NC32], F)
            nc.scalar.dma_start(bcol4[:], bcol_in[:])
            bb2 = cpool.tile([D, 256], F)
            nc.scalar.dma_start(bb2[:], bb2_in[:])
            ones128 = cpool.tile([D, 1], F)
            nc.vector.memset(ones128[:], 1.0)
            onesr = cpool.tile([1, D], F)
            nc.vector.memset(onesr[:], 1.0)

            # ---- load x directly as fp32r (same bits as fp32) ----
            xfr = [xp.tile([D, N], FR, tag=f"fx{b}", name=f"fx{b}")
                   for b in range(B_LOC)]
            for b in range(B_LOC):
                nc.sync.dma_start(xfr[b][:], xs[b])

            # ---- stat packs, column layout [p, 8b+ch] ----
            diag_pk = pk.tile([D, NC32], F)
            rsum_pk = pk.tile([D, NC32], F)
            ssq_pk = pk.tile([D, NC32], F)

            # ---- gram chunks + fused stats ----
            gt = [gsb.tile([D, N], F, tag=f"g{i}", name=f"g{i}")
                  for i in range(NC32)]
            for i in range(NC32):
                b, ch = divmod(i, NCH)
                gps = psb.tile([D, N], F, tag="big")
                lhs = xfr[b][:, 128 * ch:128 * (ch + 1)]
                nc.tensor.matmul(gps[:, 0:512], lhs, xfr[b][:, 0:512],
                                 start=True, stop=True)
                nc.tensor.matmul(gps[:, 512:1024], lhs, xfr[b][:, 512:1024],
                                 start=True, stop=True)
                # stage to SBUF + rowsum(G) in one ACT pass
                nc.scalar.activation(gt[i][:], gps[:], AF.Identity,
                                     accum_out=rsum_pk[:, i:i + 1])
                # ssq = rowsum(G*G): square on DVE into bf16 scratch (2x read
                # rate on the reduce), accumulate in fp32
                sq_scr = scr.tile([D, N], BF, tag="sqscr", name=_nm("sqscr"))
                nc.vector.tensor_tensor(sq_scr[:], gps[:], gt[i][:],
                                        op=OP.mult)
                nc.vector.reduce_sum(ssq_pk[:, i:i + 1], sq_scr[:], axis=AX.X)
                # diag = rowsum(G_block * I)
                dg_scr = scr.tile([D, D], F, tag="dgscr", name=_nm("dgscr"))
                nc.vector.tensor_tensor(dg_scr[:],
                                        gps[:, 128 * ch:128 * (ch + 1)],
                                        ident[:], op=OP.mult)
                nc.vector.reduce_sum(diag_pk[:, i:i + 1], dg_scr[:],
                                     axis=AX.X)

            # ---- derived stats (column layout) ----
            un = pk.tile([D, NC32], F)
            nc.scalar.activation(un[:], diag_pk[:], AF.Sqrt)
            uh = pk.tile([D, NC32], F)
            nc.scalar.activation(uh[:], ssq_pk[:], AF.Sqrt)
            nn_t = pk.tile([D, NC32], F)
            nc.vector.tensor_tensor(nn_t[:], uh[:], un[:], op=OP.min)
            q_pack = pk.tile([D, NC32], F)
            nc.vector.tensor_tensor(q_pack[:], nn_t[:], nn_t[:], op=OP.mult)
            ivh = pk.tile([D, NC32], F)
            nc.vector.reciprocal(ivh[:], uh[:])
            alpha = pk.tile([D, NC32], F)
            nc.vector.tensor_tensor(alpha[:], nn_t[:], ivh[:], op=OP.mult)
            rr_t = pk.tile([D, NC32], F)
            nc.vector.tensor_tensor(rr_t[:], alpha[:], rsum_pk[:], op=OP.mult)
            rr_pack = pk.tile([D, NC32], F)
            nc.vector.tensor_tensor(rr_pack[:], rr_t[:], bcol4[:], op=OP.mult)

            # ---- transpose packs to row layout, stage, AllGather ----
            stage = pk.tile([NC32, 256], F)
            qs_ps = pss.tile([NC32, D], F, tag="small", name=_nm("ps_qs"))
            nc.tensor.transpose(qs_ps[:], q_pack[:], ident[:])
            nc.scalar.copy(stage[:, 0:128], qs_ps[:])
            rs_ps = pss.tile([NC32, D], F, tag="small", name=_nm("ps_rs"))
            nc.tensor.transpose(rs_ps[:], rr_pack[:], ident[:])
            nc.scalar.copy(stage[:, 128:256], rs_ps[:])

            ag_in = dram.tile([NC32, 256], F, tag="agin")
            ag_out = dram.tile([N_CORES * NC32, 256], F, tag="agout",
                               addr_space="Shared")
            nc.sync.dma_start(ag_in[:], stage[:])
            nc.gpsimd.collective_compute(
                "AllGather", OP.bypass, replica_groups=rg,
                ins=[ag_in.opt()], outs=[ag_out.opt()])

            # ACT-table warm helper: prepone sqrt<->exp table swaps into gaps
            # where ACT is otherwise idle
            warm = cpool.tile([1, 1], F)
            nc.vector.memset(warm[:], 1.0)
            warm_o = cpool.tile([1, 1], F)

            def warm_act(func):
                nc.scalar.activation(warm_o[:], warm[:], func)

            # TT layout: [q(T0) | q(T1) | rr(T0) | rr(T1)], each (128,128)
            TT = row.tile([D, 512], F, tag="TT", name="TT")
            nc.sync.dma_start(TT[:, 0:128], ag_out[0:128, 0:128])
            nc.scalar.dma_start(TT[:, 128:256], ag_out[128:256, 0:128])
            nc.sync.dma_start(TT[:, 256:384], ag_out[0:128, 128:256])
            nc.scalar.dma_start(TT[:, 384:512], ag_out[128:256, 128:256])
            qm = TT[:, 0:256]
            rm = TT[:, 256:512]

            # ---- routing (redundant, all 32 batches) ----
            def row_t(tag):
                return row.tile([D, 256], F, tag=tag, name=_nm(tag))

            def t_chain(d):
                """global scalar t from d tile -> broadcast (128,1)"""
                part = row.tile([D, 1], F, tag="part", name=_nm("part"))
                nc.vector.reduce_sum(part[:], d[:], axis=AX.X)
                tot_ps = pss.tile([1, 1], F, tag="small", name=_nm("ps_tot"))
                nc.tensor.matmul(tot_ps[:], part[:], ones128[:],
                                 start=True, stop=True)
                tot = row.tile([1, 1], F, tag="tot", name=_nm("tot"))
                nc.scalar.copy(tot[:], tot_ps[:])
                dent = row.tile([1, 1], F, tag="dent", name=_nm("dent"))
                nc.vector.tensor_scalar(dent[:], tot[:], -0.5 / 32768.0, EPS,
                                        op0=OP.mult, op1=OP.add)
                it = row.tile([1, 1], F, tag="it", name=_nm("it"))
                nc.vector.reciprocal(it[:], dent[:])
                tv = row.tile([1, 1], F, tag="tv", name=_nm("tv"))
                nc.vector.tensor_scalar_mul(tv[:], it[:], T_NUM)
                tb_ps = pss.tile([D, 1], F, tag="small", name=_nm("ps_tb"))
                nc.tensor.matmul(tb_ps[:], onesr[:], tv[:],
                                 start=True, stop=True)
                tb = row.tile([D, 1], F, tag="tb", name=_nm("tb"))
                nc.scalar.copy(tb[:], tb_ps[:])
                return tb

            def softmax_c(d, tb):
                e = row_t("e")
                nc.scalar.activation(e[:], d[:], AF.Exp, scale=tb[:])
                warm_act(AF.Sqrt)
                pe2 = row.tile([D, 2], F, tag="pe2", name=_nm("pe2"))
                nc.vector.reduce_sum(pe2[:, 0:1], e[:, 0:128], axis=AX.X)
                nc.vector.reduce_sum(pe2[:, 1:2], e[:, 128:256], axis=AX.X)
                bs_ps = pss.tile([16, 2], F, tag="small", name=_nm("ps_bs"))
                nc.tensor.matmul(bs_ps[:], m16[:], pe2[:],
                                 start=True, stop=True)
                bs = row.tile([16, 2], F, tag="bs", name=_nm("bs"))
                nc.vector.tensor_copy(bs[:], bs_ps[:])
                binv = row.tile([16, 2], F, tag="binv", name=_nm("binv"))
                nc.vector.reciprocal(binv[:], bs[:])
                ib_ps = pss.tile([D, 2], F, tag="small", name=_nm("ps_ib"))
                nc.tensor.matmul(ib_ps[:], m16t[:], binv[:],
                                 start=True, stop=True)
                ib = row.tile([D, 2], F, tag="ib", name=_nm("ib"))
                nc.vector.tensor_copy(ib[:], ib_ps[:])
                c = row_t("c")
                nc.vector.tensor_scalar(c[:, 0:128], e[:, 0:128],
                                        ib[:, 0:1], None, op0=OP.mult)
                nc.vector.tensor_scalar(c[:, 128:256], e[:, 128:256],
                                        ib[:, 1:2], None, op0=OP.mult)
                return c

            def compute_sq_f(c):
                """sq = c*(c*q + 2rr) + bb; f = sqrt(sq)/(1+sq)"""
                u = row_t("u")
                nc.vector.tensor_tensor(u[:], c[:], qm, op=OP.mult)
                nc.vector.scalar_tensor_tensor(u[:], rm, 2.0, u[:],
                                               op0=OP.mult, op1=OP.add)
                sq = row_t("sq")
                nc.vector.tensor_tensor(sq[:], c[:], u[:], op=OP.mult)
                nc.vector.tensor_tensor(sq[:], sq[:], bb2[:], op=OP.add)
                return sq, _f_of(sq)

            def _f_of(sq):
                sqs = row_t("sqs")
                nc.scalar.activation(sqs[:], sq[:], AF.Sqrt)
                den = row_t("den")
                nc.vector.tensor_scalar_add(den[:], sq[:], 1.0)
                inv = row_t("inv")
                nc.vector.reciprocal(inv[:], den[:])
                f = row_t("f")
                nc.vector.tensor_tensor(f[:], sqs[:], inv[:], op=OP.mult)
                return f

            def compute_d(f, sq, w):
                """d = sqrt(f^2 sq + w*q - 2 f rr), w = 1-2fc precomputed"""
                a1 = row_t("a1")
                nc.vector.tensor_tensor(a1[:], f[:], sq[:], op=OP.mult)
                nc.vector.tensor_tensor(a1[:], a1[:], f[:], op=OP.mult)
                a3 = row_t("a3")
                nc.vector.tensor_tensor(a3[:], w[:], qm, op=OP.mult)
                a4 = row_t("a4")
                nc.vector.tensor_tensor(a4[:], f[:], rm, op=OP.mult)
                d2 = row_t("d2")
                nc.vector.scalar_tensor_tensor(d2[:], a4[:], -2.0, a1[:],
                                               op0=OP.mult, op1=OP.add)
                nc.vector.tensor_tensor(d2[:], d2[:], a3[:], op=OP.add)
                d = row_t("d")
                nc.scalar.activation(d[:], d2[:], AF.Sqrt)
                return d

            # iteration 0: c = C0 scalar
            sq0 = row_t("sq0")
            nc.vector.scalar_tensor_tensor(sq0[:], rm, 2.0 * C0, bb2[:],
                                           op0=OP.mult, op1=OP.add)
            nc.vector.scalar_tensor_tensor(sq0[:], qm, C0 * C0, sq0[:],
                                           op0=OP.mult, op1=OP.add)
            f0 = _f_of(sq0)
            w0 = row_t("w0")
            nc.vector.tensor_scalar(w0[:], f0[:], -2.0 * C0, 1.0,
                                    op0=OP.mult, op1=OP.add)
            d0 = compute_d(f0, sq0, w0)
            warm_act(AF.Exp)
            tb0 = t_chain(d0)
            # iteration 1
            c1 = softmax_c(d0, tb0)
            sq1, f1 = compute_sq_f(c1)
            w1 = row_t("w1")
            fc1 = row_t("fc1")
            nc.vector.tensor_tensor(fc1[:], f1[:], c1[:], op=OP.mult)
            nc.vector.tensor_scalar(w1[:], fc1[:], -2.0, 1.0,
                                    op0=OP.mult, op1=OP.add)
            d1 = compute_d(f1, sq1, w1)
            warm_act(AF.Exp)
            tb1 = t_chain(d1)
            # iteration 2 (final): only c2, f2 needed
            c2 = softmax_c(d1, tb1)
            _, f2 = compute_sq_f(c2)
            fc2 = row_t("fc2")
            nc.vector.tensor_tensor(fc2[:], f2[:], c2[:], op=OP.mult)

            # ---- extract our 4 batches to column layout via selection ----
            fcT_ps = pss.tile([D, NC32], F, tag="small", name=_nm("ps_fcT"))
            nc.tensor.matmul(fcT_ps[:], fc2[:, 0:128], sel0[:],
                             start=True, stop=False)
            nc.tensor.matmul(fcT_ps[:], fc2[:, 128:256], sel1[:],
                             start=False, stop=True)
            fcT = pk.tile([D, NC32], F)
            nc.scalar.copy(fcT[:], fcT_ps[:])
            fT_ps = pss.tile([D, NC32], F, tag="small", name=_nm("ps_fT"))
            nc.tensor.matmul(fT_ps[:], f2[:, 0:128], sel0[:],
                             start=True, stop=False)
            nc.tensor.matmul(fT_ps[:], f2[:, 128:256], sel1[:],
                             start=False, stop=True)
            fT = pk.tile([D, NC32], F)
            nc.scalar.copy(fT[:], fT_ps[:])

            a_col = pk.tile([D, NC32], F)
            nc.vector.tensor_tensor(a_col[:], fcT[:], alpha[:], op=OP.mult)
            c_col = pk.tile([D, NC32], F)
            nc.vector.tensor_tensor(c_col[:], fT[:], bcol4[:], op=OP.mult)

            # ---- output: in-place scale on staged G, stream out ----
            for i in range(NC32):
                b, ch = divmod(i, NCH)
                r = i % 8
                if r < 3:
                    nc.vector.tensor_scalar(gt[i][:], gt[i][:],
                                            a_col[:, i:i + 1],
                                            c_col[:, i:i + 1],
                                            op0=OP.mult, op1=OP.add)
                elif r < 6:
                    nc.gpsimd.tensor_scalar(gt[i][:], gt[i][:],
                                            a_col[:, i:i + 1],
                                            c_col[:, i:i + 1],
                                            op0=OP.mult, op1=OP.add)
                else:
                    nc.scalar.activation(gt[i][:], gt[i][:], AF.Identity,
                                         bias=c_col[:, i:i + 1],
                                         scale=a_col[:, i:i + 1])
                dma_eng = nc.sync if i % 2 == 0 else nc.scalar
                dma_eng.dma_start(vout[b, 128 * ch:128 * (ch + 1), :],
                                  gt[i][:])

    nc.compile()
    return nc


def _get_nc():
    global _NC_CACHE
    if _NC_CACHE is None:
        _NC_CACHE = _build()
    return _NC_CACHE


def _make_host_inputs():
    iden = np.eye(D, dtype=np.float32)
    m16 = np.zeros((D, 16), dtype=np.float32)
    m16t = np.zeros((16, D), dtype=np.float32)
    for g in range(16):
        m16[8 * g:8 * g + 8, g] = 1.0
        m16t[g, 8 * g:8 * g + 8] = 1.0
    return iden, m16, m16t


def _make_sel(core):
    sel0 = np.zeros((D, NC32), dtype=np.float32)
    sel1 = np.zeros((D, NC32), dtype=np.float32)
    for b in range(B_LOC):
        g = 4 * core + b
        for c in range(NCH):
            if g < 16:
                sel0[8 * g + c, 8 * b + c] = 1.0
            else:
                sel1[8 * (g - 16) + c, 8 * b + c] = 1.0
    return sel0, sel1


def _reference_numpy(x, bias):
    """General fallback (non-row-constant bias): straight numpy port."""
    x = x.astype(np.float32)
    bias = bias.astype(np.float32)
    u_norm = np.linalg.norm(x, axis=1)[..., None]
    u_hat = np.einsum('bdn,bdm->bnm', x, x)
    u_hat_norm = np.linalg.norm(u_hat, axis=-1, keepdims=True)
    new_norm = np.minimum(u_hat_norm, u_norm)
    u_hat = u_hat / u_hat_norm * new_norm
    t_num = np.float32(T_NUM)
    b_ij = np.zeros(u_hat.shape, dtype=np.float32)
    v_j = None
    for it in range(3):
        m = b_ij.max(axis=1, keepdims=True)
        e = np.exp(b_ij - m)
        c_ij = e / e.sum(axis=1, keepdims=True)
        s_j = c_ij * u_hat + bias
        sqn = np.sum(s_j * s_j, axis=-1, keepdims=True)
        v_j = sqn * s_j / ((1.0 + sqn) * np.sqrt(sqn))
        if it < 2:
            dd = np.linalg.norm(v_j - u_hat, axis=-1, keepdims=True)
            d_o = dd.mean()
            t = t_num / (0.5 * d_o - d_o + EPS)
            b_ij = t * dd
    return v_j


def kernel(x, bias):
    global LAST_EXEC_NS
    x = np.ascontiguousarray(x, dtype=np.float32)
    bias = np.ascontiguousarray(bias, dtype=np.float32)
    B = x.shape[0]
    row_const = bool((bias == bias[:, :, :1]).all())
    if not row_const or B != 32 or x.shape[1:] != (D, N):
        return _reference_numpy(x, bias)
    brow = np.ascontiguousarray(bias[0, :, 0]).astype(np.float32)  # (N,)
    iden, m16, m16t = _make_host_inputs()
    # bcol4[p, 8b+c] = bias[128c+p]; bb2[p', 128h+p] pattern rows repeat per 8
    bcol = brow.reshape(NCH, D).T  # (128, 8): [p, c]
    bcol4 = np.ascontiguousarray(np.tile(bcol, (1, B_LOC)))  # [p, 8b+c]? no:
    # tile gives [p, c*4] order [c0..c7,c0..c7,..]; need [8b+c] = same pattern
    # since tile repeats the 8-col block 4 times -> col index 8b+c maps to c ✓
    bb_row = (32.0 * brow) ** 2  # N*bias^2, (N,)
    bbp = bb_row.reshape(NCH, D)  # [c, p]
    bb128 = np.zeros((D, D), dtype=np.float32)  # [8g+c, p] rows repeat per 8
    for g in range(16):
        bb128[8 * g:8 * g + 8, :] = bbp
    bb2 = np.ascontiguousarray(np.concatenate([bb128, bb128], axis=1))
    nc = _get_nc()
    in_maps = []
    for core in range(N_CORES):
        sel0, sel1 = _make_sel(core)
        in_maps.append({
            "xs": np.ascontiguousarray(x[4 * core:4 * core + 4]),
            "iden": iden, "m16": m16, "m16t": m16t,
            "sel0": sel0, "sel1": sel1,
            "bcol4": bcol4, "bb2": bb2,
        })
    res = run_bass_kernel_spmd(nc, in_maps, core_ids=list(range(N_CORES)))
    LAST_EXEC_NS = res.exec_time_ns
    globals()["LAST_RES"] = res
    return np.concatenate([res.results[c]["v"] for c in range(N_CORES)], axis=0)


# revision 37
# speedup vs baseline: 1.2655x; 1.1538x over previous
"""Trainium2 Bass kernel for nn_GammaCapsGraph (capsule routing over gram matrix).

Math (per batch, X = x[b] of shape (D=128, N=1024)):
  G = X^T X (symmetric gram), u_norm = sqrt(diag G), u_hat_norm = ||G row||
  U = alpha * G rowwise, alpha = min(u_hat_norm, u_norm)/u_hat_norm
  3 routing iterations where c is a per-row scalar, so all row reductions
  collapse onto row stats:
    q[n] = min(u_hat_norm,u_norm)^2, rr[n] = alpha*bias_n*rowsum(G),
    bb[n] = N*bias_n^2       (bias verified row-constant on host)
    sq = c^2 q + 2c rr + bb;  f = sqrt(sq)/(1+sq)
    d^2 = f^2 sq + (1-2fc) q - 2f rr;  d_o = global mean(d) -> t -> c' = softmax(t d)
  Output v = (f*c*alpha) * G + (f*bias_n).

Structure (per core, 4 local batches):
  1. Gram chunks (b,ch) -> PSUM; fused ACT copy(+rowsum accum) stages G in
     SBUF; DVE tensor_tensor_reduce gives ssq (row norms^2); masked reduce
     gives diag. Stats come out in column layout packs (128, 32).
  2. Derived q/rr packs -> PE transpose -> (32, 256) stage -> ONE AllGather
     (vs two AllReduces in the serial formulation): every core gets all 32
     batches' (q, rr) and runs the routing redundantly -> t0, t1, c2, f2
     locally with no further communication.
  3. Per-core (f2*c2, f2) extracted via selection matmuls, combined with
     alpha -> per-row scale/bias; in-place scale on staged G; stream out.
A dummy AllReduce is issued first thing to pre-pay the collectives
rendezvous barrier concurrently with the gram phase.
"""
import os

import numpy as np

import concourse.bass as bass
import concourse.bacc as bacc
import concourse.tile as tile
import concourse.mybir as mybir
from concourse.bass_utils import run_bass_kernel_spmd

N_CORES = 8
B_LOC = 4
D = 128
N = 1024
NCH = 8  # column chunks of 128
NC32 = B_LOC * NCH  # 32 (b, ch) pairs
P_P = 0.9
NUM_SECONDARY = 1024
EPS = 1e-12
T_NUM = float(np.log(P_P * (NUM_SECONDARY - 1)) - np.log(1.0 - P_P))
C0 = 1.0 / N

F = mybir.dt.float32
FR = mybir.dt.float32r
BF = mybir.dt.bfloat16
AF = mybir.ActivationFunctionType
OP = mybir.AluOpType
AX = mybir.AxisListType

LAST_EXEC_NS = None
_NC_CACHE = None


def _build():
    nc = bacc.Bacc("TRN2", target_bir_lowering=False, debug=False,
                   enable_asserts=False, num_devices=N_CORES)
    xs = nc.dram_tensor("xs", (B_LOC, D, N), FR, kind="ExternalInput").ap()
    iden_in = nc.dram_tensor("iden", (D, D), F, kind="ExternalInput").ap()
    m16_in = nc.dram_tensor("m16", (D, 16), F, kind="ExternalInput").ap()
    m16t_in = nc.dram_tensor("m16t", (16, D), F, kind="ExternalInput").ap()
    sel0_in = nc.dram_tensor("sel0", (D, NC32), F, kind="ExternalInput").ap()
    sel1_in = nc.dram_tensor("sel1", (D, NC32), F, kind="ExternalInput").ap()
    bcol_in = nc.dram_tensor("bcol4", (D, NC32), F, kind="ExternalInput").ap()
    bb2_in = nc.dram_tensor("bb2", (D, 256), F, kind="ExternalInput").ap()
    vout = nc.dram_tensor("v", (B_LOC, N, N), F, kind="ExternalOutput").ap()

    rg = [list(range(N_CORES))]

    with tile.TileContext(nc) as tc:
        with (
            tc.tile_pool(name="const", bufs=1) as cpool,
            tc.tile_pool(name="xp", bufs=1) as xp,
            tc.tile_pool(name="gsb", bufs=1) as gsb,
            tc.tile_pool(name="scr", bufs=1) as scr,
            tc.tile_pool(name="pk", bufs=1) as pk,
            tc.tile_pool(name="row", bufs=1) as row,
            tc.tile_pool(name="psb", bufs=3, space="PSUM") as psb,
            tc.tile_pool(name="pss", bufs=2, space="PSUM") as pss,
            tc.tile_pool(name="dram", bufs=1, space="DRAM") as dram,
        ):
            _cnt = [0]

            def _nm(tag):
                _cnt[0] += 1
                return f"{tag}_{_cnt[0]}"

            # ---- constants ----
            ident = cpool.tile([D, D], F)
            nc.scalar.dma_start(ident[:], iden_in[:])
            m16 = cpool.tile([D, 16], F)
            nc.scalar.dma_start(m16[:], m16_in[:])
            m16t = cpool.tile([16, D], F)
            nc.scalar.dma_start(m16t[:], m16t_in[:])
            sel0 = cpool.tile([D, NC32], F)
            nc.scalar.dma_start(sel0[:], sel0_in[:])
            sel1 = cpool.tile([D, NC32], F)
            nc.scalar.dma_start(sel1[:], sel1_in[:])
            bcol4 = cpool.tile([D, NC32], F)
            nc.scalar.dma_start(bcol4[:], bcol_in[:])
            bb2 = cpool.tile([D, 256], F)
            nc.scalar.dma_start(bb2[:], bb2_in[:])
            ones128 = cpool.tile([D, 1], F)
            nc.vector.memset(ones128[:], 1.0)
            onesr = cpool.tile([1, D], F)
            nc.vector.memset(onesr[:], 1.0)

            # ---- load x directly as fp32r (same bits as fp32) ----
            xfr = [xp.tile([D, N], FR, tag=f"fx{b}", name=f"fx{b}")
                   for b in range(B_LOC)]
            for b in range(B_LOC):
                nc.sync.dma_start(xfr[b][:], xs[b])

            # ---- stat packs, column layout [p, 8b+ch] ----
            diag_pk = pk.tile([D, NC32], F)
            rsum_pk = pk.tile([D, NC32], F)
            ssq_pk = pk.tile([D, NC32], F)

            # ---- gram chunks + fused stats, derived+AG per half ----
            # Half h covers local batches {2h, 2h+1} = pack columns 16h:16h+16.
            # The derived stats + AllGather for half 0 are emitted right after
            # chunk 15 so AG-A's latency hides under half 1's gram compute.
            alpha = pk.tile([D, NC32], F)
            ag_outs = []

            def do_half(h):
                sl = slice(16 * h, 16 * h + 16)
                un = pk.tile([D, 16], F, tag=f"un{h}", name=f"un{h}")
                nc.scalar.activation(un[:], diag_pk[:, sl], AF.Sqrt)
                uh = pk.tile([D, 16], F, tag=f"uh{h}", name=f"uh{h}")
                nc.scalar.activation(uh[:], ssq_pk[:, sl], AF.Sqrt)
                nn_t = pk.tile([D, 16], F, tag=f"nn{h}", name=f"nn{h}")
                nc.vector.tensor_tensor(nn_t[:], uh[:], un[:], op=OP.min)
                q_pack = pk.tile([D, 16], F, tag=f"qp{h}", name=f"qp{h}")
                nc.vector.tensor_tensor(q_pack[:], nn_t[:], nn_t[:],
                                        op=OP.mult)
                ivh = pk.tile([D, 16], F, tag=f"ivh{h}", name=f"ivh{h}")
                nc.vector.reciprocal(ivh[:], uh[:])
                nc.vector.tensor_tensor(alpha[:, sl], nn_t[:], ivh[:],
                                        op=OP.mult)
                rr_pack = pk.tile([D, 16], F, tag=f"rp{h}", name=f"rp{h}")
                nc.vector.tensor_tensor(rr_pack[:], alpha[:, sl],
                                        rsum_pk[:, sl], op=OP.mult)
                nc.vector.tensor_tensor(rr_pack[:], rr_pack[:],
                                        bcol4[:, sl], op=OP.mult)

                stage = pk.tile([16, 256], F, tag=f"stage{h}",
                                name=f"stage{h}")
                qs_ps = pss.tile([16, D], F, tag="small", name=_nm("ps_qs"))
                nc.tensor.transpose(qs_ps[:], q_pack[:], ident[:])
                nc.scalar.copy(stage[:, 0:128], qs_ps[:])
                rs_ps = pss.tile([16, D], F, tag="small", name=_nm("ps_rs"))
                nc.tensor.transpose(rs_ps[:], rr_pack[:], ident[:])
                nc.scalar.copy(stage[:, 128:256], rs_ps[:])

                ag_in = dram.tile([16, 256], F, tag=f"agin{h}")
                ag_out = dram.tile([N_CORES * 16, 256], F, tag=f"agout{h}",
                                   addr_space="Shared")
                nc.sync.dma_start(ag_in[:], stage[:])
                nc.gpsimd.collective_compute(
                    "AllGather", OP.bypass, replica_groups=rg,
                    ins=[ag_in.opt()], outs=[ag_out.opt()])
                ag_outs.append(ag_out)

            gt = [gsb.tile([D, N], F, tag=f"g{i}", name=f"g{i}")
                  for i in range(NC32)]
            for i in range(NC32):
                b, ch = divmod(i, NCH)
                gps = psb.tile([D, N], F, tag="big")
                lhs = xfr[b][:, 128 * ch:128 * (ch + 1)]
                nc.tensor.matmul(gps[:, 0:512], lhs, xfr[b][:, 0:512],
                                 start=True, stop=True)
                nc.tensor.matmul(gps[:, 512:1024], lhs, xfr[b][:, 512:1024],
                                 start=True, stop=True)
                # stage to SBUF + rowsum(G) in one ACT pass
                nc.scalar.activation(gt[i][:], gps[:], AF.Identity,
                                     accum_out=rsum_pk[:, i:i + 1])
                # ssq = rowsum(G*G): DVE mult + reduce
                sq_scr = scr.tile([D, N], F, tag="sqscr", name=_nm("sqscr"))
                nc.vector.tensor_tensor(sq_scr[:], gps[:], gt[i][:],
                                        op=OP.mult)
                nc.vector.reduce_sum(ssq_pk[:, i:i + 1], sq_scr[:],
                                     axis=AX.X)
                # diag = rowsum(G_block * I): mask on GpSimd, reduce on DVE
                dg_scr = scr.tile([D, D], F, tag="dgscr", name=_nm("dgscr"))
                nc.gpsimd.tensor_tensor(dg_scr[:],
                                        gt[i][:, 128 * ch:128 * (ch + 1)],
                                        ident[:], op=OP.mult)
                nc.vector.reduce_sum(diag_pk[:, i:i + 1], dg_scr[:],
                                     axis=AX.X)
                if i == 15:
                    do_half(0)
            do_half(1)

            # ACT-table warm helper: prepone sqrt<->exp table swaps into gaps
            # where ACT is otherwise idle
            warm = cpool.tile([1, 1], F)
            nc.vector.memset(warm[:], 1.0)
            warm_o = cpool.tile([1, 1], F)

            def warm_act(func):
                nc.scalar.activation(warm_o[:], warm[:], func)

            # TT layout: [q(T0') | q(T1') | rr(T0') | rr(T1')]; T0' = AG-A
            # rows (all cores' local batches 0,1), T1' = AG-B rows.
            TT = row.tile([D, 512], F, tag="TT", name="TT")
            nc.sync.dma_start(TT[:, 0:128], ag_outs[0][:, 0:128])
            nc.scalar.dma_start(TT[:, 128:256], ag_outs[1][:, 0:128])
            nc.sync.dma_start(TT[:, 256:384], ag_outs[0][:, 128:256])
            nc.scalar.dma_start(TT[:, 384:512], ag_outs[1][:, 128:256])
            qm = TT[:, 0:256]
            rm = TT[:, 256:512]

            # ---- routing (redundant, all 32 batches) ----
            def row_t(tag):
                return row.tile([D, 256], F, tag=tag, name=_nm(tag))

            def t_chain(d):
                """global scalar t from d tile -> broadcast (128,1)"""
                part = row.tile([D, 1], F, tag="part", name=_nm("part"))
                nc.vector.reduce_sum(part[:], d[:], axis=AX.X)
                tot_ps = pss.tile([1, 1], F, tag="small", name=_nm("ps_tot"))
                nc.tensor.matmul(tot_ps[:], part[:], ones128[:],
                                 start=True, stop=True)
                tot = row.tile([1, 1], F, tag="tot", name=_nm("tot"))
                nc.scalar.copy(tot[:], tot_ps[:])
                dent = row.tile([1, 1], F, tag="dent", name=_nm("dent"))
                nc.vector.tensor_scalar(dent[:], tot[:], -0.5 / 32768.0, EPS,
                                        op0=OP.mult, op1=OP.add)
                it = row.tile([1, 1], F, tag="it", name=_nm("it"))
                nc.vector.reciprocal(it[:], dent[:])
                tv = row.tile([1, 1], F, tag="tv", name=_nm("tv"))
                nc.vector.tensor_scalar_mul(tv[:], it[:], T_NUM)
                tb_ps = pss.tile([D, 1], F, tag="small", name=_nm("ps_tb"))
                nc.tensor.matmul(tb_ps[:], onesr[:], tv[:],
                                 start=True, stop=True)
                tb = row.tile([D, 1], F, tag="tb", name=_nm("tb"))
                nc.scalar.copy(tb[:], tb_ps[:])
                return tb

            def softmax_c(d, tb):
                e = row_t("e")
                nc.scalar.activation(e[:], d[:], AF.Exp, scale=tb[:])
                warm_act(AF.Sqrt)
                pe2 = row.tile([D, 2], F, tag="pe2", name=_nm("pe2"))
                nc.vector.reduce_sum(pe2[:, 0:1], e[:, 0:128], axis=AX.X)
                nc.vector.reduce_sum(pe2[:, 1:2], e[:, 128:256], axis=AX.X)
                bs_ps = pss.tile([16, 2], F, tag="small", name=_nm("ps_bs"))
                nc.tensor.matmul(bs_ps[:], m16[:], pe2[:],
                                 start=True, stop=True)
                bs = row.tile([16, 2], F, tag="bs", name=_nm("bs"))
                nc.vector.tensor_copy(bs[:], bs_ps[:])
                binv = row.tile([16, 2], F, tag="binv", name=_nm("binv"))
                nc.vector.reciprocal(binv[:], bs[:])
                ib_ps = pss.tile([D, 2], F, tag="small", name=_nm("ps_ib"))
                nc.tensor.matmul(ib_ps[:], m16t[:], binv[:],
                                 start=True, stop=True)
                ib = row.tile([D, 2], F, tag="ib", name=_nm("ib"))
                nc.vector.tensor_copy(ib[:], ib_ps[:])
                c = row_t("c")
                nc.vector.tensor_scalar(c[:, 0:128], e[:, 0:128],
                                        ib[:, 0:1], None, op0=OP.mult)
                nc.vector.tensor_scalar(c[:, 128:256], e[:, 128:256],
                                        ib[:, 1:2], None, op0=OP.mult)
                return c

            def compute_sq_f(c):
                """sq = c*(c*q + 2rr) + bb; f = sqrt(sq)/(1+sq)"""
                u = row_t("u")
                nc.vector.tensor_tensor(u[:], c[:], qm, op=OP.mult)
                nc.vector.scalar_tensor_tensor(u[:], rm, 2.0, u[:],
                                               op0=OP.mult, op1=OP.add)
                sq = row_t("sq")
                nc.vector.tensor_tensor(sq[:], c[:], u[:], op=OP.mult)
                nc.vector.tensor_tensor(sq[:], sq[:], bb2[:], op=OP.add)
                return sq, _f_of(sq)

            def _f_of(sq):
                sqs = row_t("sqs")
                nc.scalar.activation(sqs[:], sq[:], AF.Sqrt)
                den = row_t("den")
                nc.vector.tensor_scalar_add(den[:], sq[:], 1.0)
                inv = row_t("inv")
                nc.vector.reciprocal(inv[:], den[:])
                f = row_t("f")
                nc.vector.tensor_tensor(f[:], sqs[:], inv[:], op=OP.mult)
                return f

            def compute_d(f, sq, w):
                """d = sqrt(f^2 sq + w*q - 2 f rr), w = 1-2fc precomputed"""
                a1 = row_t("a1")
                nc.vector.tensor_tensor(a1[:], f[:], sq[:], op=OP.mult)
                nc.vector.tensor_tensor(a1[:], a1[:], f[:], op=OP.mult)
                a3 = row_t("a3")
                nc.vector.tensor_tensor(a3[:], w[:], qm, op=OP.mult)
                a4 = row_t("a4")
                nc.vector.tensor_tensor(a4[:], f[:], rm, op=OP.mult)
                d2 = row_t("d2")
                nc.vector.scalar_tensor_tensor(d2[:], a4[:], -2.0, a1[:],
                                               op0=OP.mult, op1=OP.add)
                nc.vector.tensor_tensor(d2[:], d2[:], a3[:], op=OP.add)
                d = row_t("d")
                nc.scalar.activation(d[:], d2[:], AF.Sqrt)
                return d

            # iteration 0: c = C0 scalar
            sq0 = row_t("sq0")
            nc.vector.scalar_tensor_tensor(sq0[:], rm, 2.0 * C0, bb2[:],
                                           op0=OP.mult, op1=OP.add)
            nc.vector.scalar_tensor_tensor(sq0[:], qm, C0 * C0, sq0[:],
                                           op0=OP.mult, op1=OP.add)
            f0 = _f_of(sq0)
            w0 = row_t("w0")
            nc.vector.tensor_scalar(w0[:], f0[:], -2.0 * C0, 1.0,
                                    op0=OP.mult, op1=OP.add)
            d0 = compute_d(f0, sq0, w0)
            warm_act(AF.Exp)
            tb0 = t_chain(d0)
            # iteration 1
            c1 = softmax_c(d0, tb0)
            sq1, f1 = compute_sq_f(c1)
            w1 = row_t("w1")
            fc1 = row_t("fc1")
            nc.vector.tensor_tensor(fc1[:], f1[:], c1[:], op=OP.mult)
            nc.vector.tensor_scalar(w1[:], fc1[:], -2.0, 1.0,
                                    op0=OP.mult, op1=OP.add)
            d1 = compute_d(f1, sq1, w1)
            warm_act(AF.Exp)
            tb1 = t_chain(d1)
            # iteration 2 (final): only c2, f2 needed
            c2 = softmax_c(d1, tb1)
            _, f2 = compute_sq_f(c2)
            fc2 = row_t("fc2")
            nc.vector.tensor_tensor(fc2[:], f2[:], c2[:], op=OP.mult)

            # ---- extract our 4 batches to column layout via selection ----
            fcT_ps = pss.tile([D, NC32], F, tag="small", name=_nm("ps_fcT"))
            nc.tensor.matmul(fcT_ps[:], fc2[:, 0:128], sel0[:],
                             start=True, stop=False)
            nc.tensor.matmul(fcT_ps[:], fc2[:, 128:256], sel1[:],
                             start=False, stop=True)
            fcT = pk.tile([D, NC32], F)
            nc.scalar.copy(fcT[:], fcT_ps[:])
            fT_ps = pss.tile([D, NC32], F, tag="small", name=_nm("ps_fT"))
            nc.tensor.matmul(fT_ps[:], f2[:, 0:128], sel0[:],
                             start=True, stop=False)
            nc.tensor.matmul(fT_ps[:], f2[:, 128:256], sel1[:],
                             start=False, stop=True)
            fT = pk.tile([D, NC32], F)
            nc.scalar.copy(fT[:], fT_ps[:])

            a_col = pk.tile([D, NC32], F)
            nc.vector.tensor_tensor(a_col[:], fcT[:], alpha[:], op=OP.mult)
            c_col = pk.tile([D, NC32], F)
            nc.vector.tensor_tensor(c_col[:], fT[:], bcol4[:], op=OP.mult)

            # ---- output: in-place scale on staged G, stream out ----
            for i in range(NC32):
                b, ch = divmod(i, NCH)
                if i % 2 == 0:
                    nc.vector.tensor_scalar(gt[i][:], gt[i][:],
                                            a_col[:, i:i + 1],
                                            c_col[:, i:i + 1],
                                            op0=OP.mult, op1=OP.add)
                else:
                    nc.scalar.activation(gt[i][:], gt[i][:], AF.Identity,
                                         bias=c_col[:, i:i + 1],
                                         scale=a_col[:, i:i + 1])
                dma_eng = nc.sync if i % 2 == 0 else nc.scalar
                dma_eng.dma_start(vout[b, 128 * ch:128 * (ch + 1), :],
                                  gt[i][:])

    nc.compile()
    return nc


def _get_nc():
    global _NC_CACHE
    if _NC_CACHE is None:
        _NC_CACHE = _build()
    return _NC_CACHE


def _make_host_inputs():
    iden = np.eye(D, dtype=np.float32)
    m16 = np.zeros((D, 16), dtype=np.float32)
    m16t = np.zeros((16, D), dtype=np.float32)
    for g in range(16):
        m16[8 * g:8 * g + 8, g] = 1.0
        m16t[g, 8 * g:8 * g + 8] = 1.0
    return iden, m16, m16t


def _make_sel(core):
    # T0' rows (AG-A): rank r block at 16r, local batches 0,1.
    # T1' rows (AG-B): rank r block at 16r, local batches 2,3.
    sel0 = np.zeros((D, NC32), dtype=np.float32)
    sel1 = np.zeros((D, NC32), dtype=np.float32)
    for b in range(B_LOC):
        for c in range(NCH):
            if b < 2:
                sel0[16 * core + 8 * b + c, 8 * b + c] = 1.0
            else:
                sel1[16 * core + 8 * (b - 2) + c, 8 * b + c] = 1.0
    return sel0, sel1


def _reference_numpy(x, bias):
    """General fallback (non-row-constant bias): straight numpy port."""
    x = x.astype(np.float32)
    bias = bias.astype(np.float32)
    u_norm = np.linalg.norm(x, axis=1)[..., None]
    u_hat = np.einsum('bdn,bdm->bnm', x, x)
    u_hat_norm = np.linalg.norm(u_hat, axis=-1, keepdims=True)
    new_norm = np.minimum(u_hat_norm, u_norm)
    u_hat = u_hat / u_hat_norm * new_norm
    t_num = np.float32(T_NUM)
    b_ij = np.zeros(u_hat.shape, dtype=np.float32)
    v_j = None
    for it in range(3):
        m = b_ij.max(axis=1, keepdims=True)
        e = np.exp(b_ij - m)
        c_ij = e / e.sum(axis=1, keepdims=True)
        s_j = c_ij * u_hat + bias
        sqn = np.sum(s_j * s_j, axis=-1, keepdims=True)
        v_j = sqn * s_j / ((1.0 + sqn) * np.sqrt(sqn))
        if it < 2:
            dd = np.linalg.norm(v_j - u_hat, axis=-1, keepdims=True)
            d_o = dd.mean()
            t = t_num / (0.5 * d_o - d_o + EPS)
            b_ij = t * dd
    return v_j


def kernel(x, bias):
    global LAST_EXEC_NS
    x = np.ascontiguousarray(x, dtype=np.float32)
    bias = np.ascontiguousarray(bias, dtype=np.float32)
    B = x.shape[0]
    row_const = bool((bias == bias[:, :, :1]).all())
    if not row_const or B != 32 or x.shape[1:] != (D, N):
        return _reference_numpy(x, bias)
    brow = np.ascontiguousarray(bias[0, :, 0]).astype(np.float32)  # (N,)
    iden, m16, m16t = _make_host_inputs()
    # bcol4[p, 8b+c] = bias[128c+p]; bb2[p', 128h+p] pattern rows repeat per 8
    bcol = brow.reshape(NCH, D).T  # (128, 8): [p, c]
    bcol4 = np.ascontiguousarray(np.tile(bcol, (1, B_LOC)))  # [p, 8b+c]? no:
    # tile gives [p, c*4] order [c0..c7,c0..c7,..]; need [8b+c] = same pattern
    # since tile repeats the 8-col block 4 times -> col index 8b+c maps to c ✓
    bb_row = (32.0 * brow) ** 2  # N*bias^2, (N,)
    bbp = bb_row.reshape(NCH, D)  # [c, p]
    bb128 = np.zeros((D, D), dtype=np.float32)  # [8g+c, p] rows repeat per 8
    for g in range(16):
        bb128[8 * g:8 * g + 8, :] = bbp
    bb2 = np.ascontiguousarray(np.concatenate([bb128, bb128], axis=1))
    nc = _get_nc()
    in_maps = []
    for core in range(N_CORES):
        sel0, sel1 = _make_sel(core)
        in_maps.append({
            "xs": np.ascontiguousarray(x[4 * core:4 * core + 4]),
            "iden": iden, "m16": m16, "m16t": m16t,
            "sel0": sel0, "sel1": sel1,
            "bcol4": bcol4, "bb2": bb2,
        })
    res = run_bass_kernel_spmd(nc, in_maps, core_ids=list(range(N_CORES)))
    LAST_EXEC_NS = res.exec_time_ns
    globals()["LAST_RES"] = res
    return np.concatenate([res.results[c]["v"] for c in range(N_CORES)], axis=0)


# revision 43
# speedup vs baseline: 1.4093x; 1.1136x over previous
"""Trainium2 Bass kernel for nn_GammaCapsGraph (capsule routing over gram matrix).

Math (per batch, X = x[b] of shape (D=128, N=1024)):
  G = X^T X (symmetric gram), u_norm = sqrt(diag G), u_hat_norm = ||G row||
  U = alpha * G rowwise, alpha = min(u_hat_norm, u_norm)/u_hat_norm
  3 routing iterations where c is a per-row scalar, so all row reductions
  collapse onto row stats:
    q[n] = min(u_hat_norm,u_norm)^2, rr[n] = alpha*bias_n*rowsum(G),
    bb[n] = N*bias_n^2       (bias verified row-constant on host)
    sq = c^2 q + 2c rr + bb;  f = sqrt(sq)/(1+sq)
    d^2 = f^2 sq + (1-2fc) q - 2f rr;  d_o = global mean(d) -> t -> c' = softmax(t d)
  Output v = (f*c*alpha) * G + (f*bias_n).

Structure (per core, 4 local batches):
  1. Gram chunks (b,ch) -> PSUM; fused ACT copy(+rowsum accum) stages G in
     SBUF; DVE tensor_tensor_reduce gives ssq (row norms^2); masked reduce
     gives diag. Stats come out in column layout packs (128, 32).
  2. Derived q/rr packs -> PE transpose -> (32, 256) stage -> ONE AllGather
     (vs two AllReduces in the serial formulation): every core gets all 32
     batches' (q, rr) and runs the routing redundantly -> t0, t1, c2, f2
     locally with no further communication.
  3. Per-core (f2*c2, f2) extracted via selection matmuls, combined with
     alpha -> per-row scale/bias; in-place scale on staged G; stream out.
A dummy AllReduce is issued first thing to pre-pay the collectives
rendezvous barrier concurrently with the gram phase.
"""
import os

import numpy as np

import concourse.bass as bass
import concourse.bacc as bacc
import concourse.tile as tile
import concourse.mybir as mybir
from concourse.bass_utils import run_bass_kernel_spmd

N_CORES = 8
B_LOC = 4
D = 128
N = 1024
NCH = 8  # column chunks of 128
NC32 = B_LOC * NCH  # 32 (b, ch) pairs
P_P = 0.9
NUM_SECONDARY = 1024
EPS = 1e-12
T_NUM = float(np.log(P_P * (NUM_SECONDARY - 1)) - np.log(1.0 - P_P))
C0 = 1.0 / N

F = mybir.dt.float32
FR = mybir.dt.float32r
BF = mybir.dt.bfloat16
AF = mybir.ActivationFunctionType
OP = mybir.AluOpType
AX = mybir.AxisListType

LAST_EXEC_NS = None
_NC_CACHE = None


def _build():
    nc = bacc.Bacc("TRN2", target_bir_lowering=False, debug=False,
                   enable_asserts=False, num_devices=N_CORES)
    xs = nc.dram_tensor("xs", (B_LOC, D, N), FR, kind="ExternalInput").ap()
    iden_in = nc.dram_tensor("iden", (D, D), F, kind="ExternalInput").ap()
    m16_in = nc.dram_tensor("m16", (D, 16), F, kind="ExternalInput").ap()
    m16t_in = nc.dram_tensor("m16t", (16, D), F, kind="ExternalInput").ap()
    sel0_in = nc.dram_tensor("sel0", (D, NC32), F, kind="ExternalInput").ap()
    sel1_in = nc.dram_tensor("sel1", (D, NC32), F, kind="ExternalInput").ap()
    bcol_in = nc.dram_tensor("bcol4", (D, NC32), F, kind="ExternalInput").ap()
    bb2_in = nc.dram_tensor("bb2", (D, 256), F, kind="ExternalInput").ap()
    vout = nc.dram_tensor("v", (B_LOC, N, N), F, kind="ExternalOutput").ap()

    rg = [list(range(N_CORES))]

    with tile.TileContext(nc) as tc:
        with (
            tc.tile_pool(name="const", bufs=1) as cpool,
            tc.tile_pool(name="xp", bufs=1) as xp,
            tc.tile_pool(name="gsb", bufs=1) as gsb,
            tc.tile_pool(name="scr", bufs=2) as scr,
            tc.tile_pool(name="pk", bufs=1) as pk,
            tc.tile_pool(name="row", bufs=1) as row,
            tc.tile_pool(name="psb", bufs=3, space="PSUM") as psb,
            tc.tile_pool(name="pss", bufs=2, space="PSUM") as pss,
            tc.tile_pool(name="dram", bufs=1, space="DRAM") as dram,
        ):
            _cnt = [0]

            def _nm(tag):
                _cnt[0] += 1
                return f"{tag}_{_cnt[0]}"

            # ---- constants ----
            ident = cpool.tile([D, D], F)
            nc.scalar.dma_start(ident[:], iden_in[:])
            m16 = cpool.tile([D, 16], F)
            nc.scalar.dma_start(m16[:], m16_in[:])
            m16t = cpool.tile([16, D], F)
            nc.scalar.dma_start(m16t[:], m16t_in[:])
            sel0 = cpool.tile([D, NC32], F)
            nc.scalar.dma_start(sel0[:], sel0_in[:])
            sel1 = cpool.tile([D, NC32], F)
            nc.scalar.dma_start(sel1[:], sel1_in[:])
            bcol4 = cpool.tile([D, NC32], F)
            nc.scalar.dma_start(bcol4[:], bcol_in[:])
            bb2 = cpool.tile([D, 256], F)
            nc.scalar.dma_start(bb2[:], bb2_in[:])
            ones128 = cpool.tile([D, 1], F)
            nc.vector.memset(ones128[:], 1.0)
            onesr = cpool.tile([1, D], F)
            nc.vector.memset(onesr[:], 1.0)

            # ---- load x directly as fp32r (same bits as fp32) ----
            xfr = [xp.tile([D, N], FR, tag=f"fx{b}", name=f"fx{b}")
                   for b in range(B_LOC)]
            for b in range(B_LOC):
                nc.sync.dma_start(xfr[b][:], xs[b])

            # ---- stat packs, column layout [p, 8b+ch] ----
            diag_pk = pk.tile([D, NC32], F)
            rsum_pk = pk.tile([D, NC32], F)
            ssq_pk = pk.tile([D, NC32], F)

            # ---- gram chunks + fused stats, derived+AG per half ----
            # Half h covers local batches {2h, 2h+1} = pack columns 16h:16h+16.
            # The derived stats + AllGather for half 0 are emitted right after
            # chunk 15 so AG-A's latency hides under half 1's gram compute.
            alpha = pk.tile([D, NC32], F)
            ag_outs = []
            # bb in column layout (128,16): 1024*bias^2, same for both halves
            bbc = pk.tile([D, 16], F)
            nc.scalar.activation(bbc[:], bcol4[:, 0:16], AF.Square,
                                 scale=32.0)

            def do_half(h):
                sl = slice(16 * h, 16 * h + 16)
                un = pk.tile([D, 16], F, tag=f"un{h}", name=f"un{h}")
                nc.scalar.activation(un[:], diag_pk[:, sl], AF.Sqrt)
                uh = pk.tile([D, 16], F, tag=f"uh{h}", name=f"uh{h}")
                nc.scalar.activation(uh[:], ssq_pk[:, sl], AF.Sqrt)
                nn_t = pk.tile([D, 16], F, tag=f"nn{h}", name=f"nn{h}")
                nc.vector.tensor_tensor(nn_t[:], uh[:], un[:], op=OP.min)
                q_pack = pk.tile([D, 16], F, tag=f"qp{h}", name=f"qp{h}")
                nc.vector.tensor_tensor(q_pack[:], nn_t[:], nn_t[:],
                                        op=OP.mult)
                ivh = pk.tile([D, 16], F, tag=f"ivh{h}", name=f"ivh{h}")
                nc.vector.reciprocal(ivh[:], uh[:])
                nc.vector.tensor_tensor(alpha[:, sl], nn_t[:], ivh[:],
                                        op=OP.mult)
                rr_pack = pk.tile([D, 16], F, tag=f"rp{h}", name=f"rp{h}")
                nc.vector.tensor_tensor(rr_pack[:], alpha[:, sl],
                                        rsum_pk[:, sl], op=OP.mult)
                nc.vector.tensor_tensor(rr_pack[:], rr_pack[:],
                                        bcol4[:, sl], op=OP.mult)

                # iteration 0 (c = 1/N) locally in column layout: d0
                sqh = pk.tile([D, 16], F, tag=f"sqh{h}", name=f"sqh{h}")
                nc.vector.scalar_tensor_tensor(sqh[:], rr_pack[:], 2.0 * C0,
                                               bbc[:], op0=OP.mult,
                                               op1=OP.add)
                nc.vector.scalar_tensor_tensor(sqh[:], q_pack[:], C0 * C0,
                                               sqh[:], op0=OP.mult,
                                               op1=OP.add)
                sqsh = pk.tile([D, 16], F, tag=f"sqsh{h}", name=f"sqsh{h}")
                nc.scalar.activation(sqsh[:], sqh[:], AF.Sqrt)
                denh = pk.tile([D, 16], F, tag=f"denh{h}", name=f"denh{h}")
                nc.vector.tensor_scalar_add(denh[:], sqh[:], 1.0)
                invh = pk.tile([D, 16], F, tag=f"invh{h}", name=f"invh{h}")
                nc.vector.reciprocal(invh[:], denh[:])
                fh = pk.tile([D, 16], F, tag=f"fh{h}", name=f"fh{h}")
                nc.vector.tensor_tensor(fh[:], sqsh[:], invh[:], op=OP.mult)
                a1h = pk.tile([D, 16], F, tag=f"a1h{h}", name=f"a1h{h}")
                nc.vector.tensor_tensor(a1h[:], fh[:], sqh[:], op=OP.mult)
                nc.vector.tensor_tensor(a1h[:], a1h[:], fh[:], op=OP.mult)
                wh = pk.tile([D, 16], F, tag=f"wh{h}", name=f"wh{h}")
                nc.vector.tensor_scalar(wh[:], fh[:], -2.0 * C0, 1.0,
                                        op0=OP.mult, op1=OP.add)
                a3h = pk.tile([D, 16], F, tag=f"a3h{h}", name=f"a3h{h}")
                nc.vector.tensor_tensor(a3h[:], wh[:], q_pack[:], op=OP.mult)
                a4h = pk.tile([D, 16], F, tag=f"a4h{h}", name=f"a4h{h}")
                nc.vector.tensor_tensor(a4h[:], fh[:], rr_pack[:],
                                        op=OP.mult)
                d2h = pk.tile([D, 16], F, tag=f"d2h{h}", name=f"d2h{h}")
                nc.vector.scalar_tensor_tensor(d2h[:], a4h[:], -2.0, a1h[:],
                                               op0=OP.mult, op1=OP.add)
                nc.vector.tensor_tensor(d2h[:], d2h[:], a3h[:], op=OP.add)
                d0h = pk.tile([D, 16], F, tag=f"d0h{h}", name=f"d0h{h}")
                nc.scalar.activation(d0h[:], d2h[:], AF.Sqrt)

                stage = pk.tile([16, 384], F, tag=f"stage{h}",
                                name=f"stage{h}")
                qs_ps = pss.tile([16, D], F, tag="small", name=_nm("ps_qs"))
                nc.tensor.transpose(qs_ps[:], q_pack[:], ident[:])
                nc.scalar.copy(stage[:, 0:128], qs_ps[:])
                rs_ps = pss.tile([16, D], F, tag="small", name=_nm("ps_rs"))
                nc.tensor.transpose(rs_ps[:], rr_pack[:], ident[:])
                nc.scalar.copy(stage[:, 128:256], rs_ps[:])
                ds_ps = pss.tile([16, D], F, tag="small", name=_nm("ps_ds"))
                nc.tensor.transpose(ds_ps[:], d0h[:], ident[:])
                nc.scalar.copy(stage[:, 256:384], ds_ps[:])

                ag_in = dram.tile([16, 384], F, tag=f"agin{h}")
                ag_out = dram.tile([N_CORES * 16, 384], F, tag=f"agout{h}",
                                   addr_space="Shared")
                nc.sync.dma_start(ag_in[:], stage[:])
                nc.gpsimd.collective_compute(
                    "AllGather", OP.bypass, replica_groups=rg,
                    ins=[ag_in.opt()], outs=[ag_out.opt()])
                ag_outs.append(ag_out)

            gt = [gsb.tile([D, N], F, tag=f"g{i}", name=f"g{i}")
                  for i in range(NC32)]
            for i in range(NC32):
                b, ch = divmod(i, NCH)
                gps = psb.tile([D, N], F, tag="big")
                lhs = xfr[b][:, 128 * ch:128 * (ch + 1)]
                nc.tensor.matmul(gps[:, 0:512], lhs, xfr[b][:, 0:512],
                                 start=True, stop=True)
                nc.tensor.matmul(gps[:, 512:1024], lhs, xfr[b][:, 512:1024],
                                 start=True, stop=True)
                # stage to SBUF + rowsum(G) in one ACT pass
                nc.scalar.activation(gt[i][:], gps[:], AF.Identity,
                                     accum_out=rsum_pk[:, i:i + 1])
                # ssq = rowsum(G*G): the square alternates DVE (from PSUM) /
                # GpSimd (from the SBUF copy) to balance engine load; the
                # free-dim reduce only exists on DVE
                sq_scr = scr.tile([D, N], F, tag="sqscr", name=_nm("sqscr"))
                if i % 2 == 0:
                    nc.vector.tensor_tensor(sq_scr[:], gps[:], gt[i][:],
                                            op=OP.mult)
                else:
                    nc.gpsimd.tensor_tensor(sq_scr[:], gt[i][:], gt[i][:],
                                            op=OP.mult)
                nc.vector.reduce_sum(ssq_pk[:, i:i + 1], sq_scr[:],
                                     axis=AX.X)
                # diag = rowsum(G_block * I): mask on GpSimd, reduce on DVE
                dg_scr = scr.tile([D, D], F, tag="dgscr", name=_nm("dgscr"))
                nc.gpsimd.tensor_tensor(dg_scr[:],
                                        gt[i][:, 128 * ch:128 * (ch + 1)],
                                        ident[:], op=OP.mult)
                nc.vector.reduce_sum(diag_pk[:, i:i + 1], dg_scr[:],
                                     axis=AX.X)
                if i == 15:
                    do_half(0)
            do_half(1)

            # ACT-table warm helper: prepone sqrt<->exp table swaps into gaps
            # where ACT is otherwise idle
            warm = cpool.tile([1, 1], F)
            nc.vector.memset(warm[:], 1.0)
            warm_o = cpool.tile([1, 1], F)

            def warm_act(func):
                nc.scalar.activation(warm_o[:], warm[:], func)

            # TT layout: [q(T0')|q(T1') | rr(T0')|rr(T1') | d0(T0')|d0(T1')];
            # T0' = AG-A rows (all cores' local batches 0,1), T1' = AG-B rows.
            TT = row.tile([D, 768], F, tag="TT", name="TT")
            nc.sync.dma_start(TT[:, 0:128], ag_outs[0][:, 0:128])
            nc.scalar.dma_start(TT[:, 128:256], ag_outs[1][:, 0:128])
            nc.sync.dma_start(TT[:, 256:384], ag_outs[0][:, 128:256])
            nc.scalar.dma_start(TT[:, 384:512], ag_outs[1][:, 128:256])
            nc.sync.dma_start(TT[:, 512:640], ag_outs[0][:, 256:384])
            nc.scalar.dma_start(TT[:, 640:768], ag_outs[1][:, 256:384])
            qm = TT[:, 0:256]
            rm = TT[:, 256:512]
            d0m = TT[:, 512:768]

            # ---- routing (redundant, all 32 batches) ----
            def row_t(tag):
                return row.tile([D, 256], F, tag=tag, name=_nm(tag))

            def t_chain(d_ap):
                """global scalar t from d AP -> broadcast (128,1)"""
                part = row.tile([D, 1], F, tag="part", name=_nm("part"))
                nc.vector.reduce_sum(part[:], d_ap, axis=AX.X)
                tot_ps = pss.tile([1, 1], F, tag="small", name=_nm("ps_tot"))
                nc.tensor.matmul(tot_ps[:], part[:], ones128[:],
                                 start=True, stop=True)
                tot = row.tile([1, 1], F, tag="tot", name=_nm("tot"))
                nc.scalar.copy(tot[:], tot_ps[:])
                dent = row.tile([1, 1], F, tag="dent", name=_nm("dent"))
                nc.vector.tensor_scalar(dent[:], tot[:], -0.5 / 32768.0, EPS,
                                        op0=OP.mult, op1=OP.add)
                it = row.tile([1, 1], F, tag="it", name=_nm("it"))
                nc.vector.reciprocal(it[:], dent[:])
                tv = row.tile([1, 1], F, tag="tv", name=_nm("tv"))
                nc.vector.tensor_scalar_mul(tv[:], it[:], T_NUM)
                tb_ps = pss.tile([D, 1], F, tag="small", name=_nm("ps_tb"))
                nc.tensor.matmul(tb_ps[:], onesr[:], tv[:],
                                 start=True, stop=True)
                tb = row.tile([D, 1], F, tag="tb", name=_nm("tb"))
                nc.scalar.copy(tb[:], tb_ps[:])
                return tb

            def softmax_c(d_ap, tb):
                e = row_t("e")
                nc.scalar.activation(e[:], d_ap, AF.Exp, scale=tb[:])
                warm_act(AF.Sqrt)
                pe2 = row.tile([D, 2], F, tag="pe2", name=_nm("pe2"))
                nc.vector.reduce_sum(pe2[:, 0:1], e[:, 0:128], axis=AX.X)
                nc.vector.reduce_sum(pe2[:, 1:2], e[:, 128:256], axis=AX.X)
                bs_ps = pss.tile([16, 2], F, tag="small", name=_nm("ps_bs"))
                nc.tensor.matmul(bs_ps[:], m16[:], pe2[:],
                                 start=True, stop=True)
                bs = row.tile([16, 2], F, tag="bs", name=_nm("bs"))
                nc.vector.tensor_copy(bs[:], bs_ps[:])
                binv = row.tile([16, 2], F, tag="binv", name=_nm("binv"))
                nc.vector.reciprocal(binv[:], bs[:])
                ib_ps = pss.tile([D, 2], F, tag="small", name=_nm("ps_ib"))
                nc.tensor.matmul(ib_ps[:], m16t[:], binv[:],
                                 start=True, stop=True)
                ib = row.tile([D, 2], F, tag="ib", name=_nm("ib"))
                nc.vector.tensor_copy(ib[:], ib_ps[:])
                c = row_t("c")
                nc.vector.tensor_scalar(c[:, 0:128], e[:, 0:128],
                                        ib[:, 0:1], None, op0=OP.mult)
                nc.vector.tensor_scalar(c[:, 128:256], e[:, 128:256],
                                        ib[:, 1:2], None, op0=OP.mult)
                return c

            def compute_sq_f(c):
                """sq = c*(c*q + 2rr) + bb; f = sqrt(sq)/(1+sq)"""
                u = row_t("u")
                nc.vector.tensor_tensor(u[:], c[:], qm, op=OP.mult)
                nc.vector.scalar_tensor_tensor(u[:], rm, 2.0, u[:],
                                               op0=OP.mult, op1=OP.add)
                sq = row_t("sq")
                nc.vector.tensor_tensor(sq[:], c[:], u[:], op=OP.mult)
                nc.vector.tensor_tensor(sq[:], sq[:], bb2[:], op=OP.add)
                return sq, _f_of(sq)

            def _f_of(sq):
                sqs = row_t("sqs")
                nc.scalar.activation(sqs[:], sq[:], AF.Sqrt)
                den = row_t("den")
                nc.vector.tensor_scalar_add(den[:], sq[:], 1.0)
                inv = row_t("inv")
                nc.vector.reciprocal(inv[:], den[:])
                f = row_t("f")
                nc.vector.tensor_tensor(f[:], sqs[:], inv[:], op=OP.mult)
                return f

            def compute_d(f, sq, w):
                """d = sqrt(f^2 sq + w*q - 2 f rr), w = 1-2fc precomputed"""
                a1 = row_t("a1")
                nc.vector.tensor_tensor(a1[:], f[:], sq[:], op=OP.mult)
                nc.vector.tensor_tensor(a1[:], a1[:], f[:], op=OP.mult)
                a3 = row_t("a3")
                nc.vector.tensor_tensor(a3[:], w[:], qm, op=OP.mult)
                a4 = row_t("a4")
                nc.vector.tensor_tensor(a4[:], f[:], rm, op=OP.mult)
                d2 = row_t("d2")
                nc.vector.scalar_tensor_tensor(d2[:], a4[:], -2.0, a1[:],
                                               op0=OP.mult, op1=OP.add)
                nc.vector.tensor_tensor(d2[:], d2[:], a3[:], op=OP.add)
                d = row_t("d")
                nc.scalar.activation(d[:], d2[:], AF.Sqrt)
                return d

            # iteration 0 was computed per-half before the gather; d0 arrives
            # via the AllGather in row layout
            warm_act(AF.Exp)
            tb0 = t_chain(d0m)
            # iteration 1
            c1 = softmax_c(d0m, tb0)
            sq1, f1 = compute_sq_f(c1)
            w1 = row_t("w1")
            fc1 = row_t("fc1")
            nc.vector.tensor_tensor(fc1[:], f1[:], c1[:], op=OP.mult)
            nc.vector.tensor_scalar(w1[:], fc1[:], -2.0, 1.0,
                                    op0=OP.mult, op1=OP.add)
            d1 = compute_d(f1, sq1, w1)
            warm_act(AF.Exp)
            tb1 = t_chain(d1[:])
            # iteration 2 (final): only c2, f2 needed
            c2 = softmax_c(d1[:], tb1)
            _, f2 = compute_sq_f(c2)
            fc2 = row_t("fc2")
            nc.vector.tensor_tensor(fc2[:], f2[:], c2[:], op=OP.mult)

            # ---- extract our 4 batches to column layout via selection ----
            fcT_ps = pss.tile([D, NC32], F, tag="small", name=_nm("ps_fcT"))
            nc.tensor.matmul(fcT_ps[:], fc2[:, 0:128], sel0[:],
                             start=True, stop=False)
            nc.tensor.matmul(fcT_ps[:], fc2[:, 128:256], sel1[:],
                             start=False, stop=True)
            fcT = pk.tile([D, NC32], F)
            nc.scalar.copy(fcT[:], fcT_ps[:])
            fT_ps = pss.tile([D, NC32], F, tag="small", name=_nm("ps_fT"))
            nc.tensor.matmul(fT_ps[:], f2[:, 0:128], sel0[:],
                             start=True, stop=False)
            nc.tensor.matmul(fT_ps[:], f2[:, 128:256], sel1[:],
                             start=False, stop=True)
            fT = pk.tile([D, NC32], F)
            nc.scalar.copy(fT[:], fT_ps[:])

            a_col = pk.tile([D, NC32], F)
            nc.vector.tensor_tensor(a_col[:], fcT[:], alpha[:], op=OP.mult)
            c_col = pk.tile([D, NC32], F)
            nc.vector.tensor_tensor(c_col[:], fT[:], bcol4[:], op=OP.mult)

            # ---- output: in-place scale on staged G, stream out ----
            for i in range(NC32):
                b, ch = divmod(i, NCH)
                if i % 2 == 0:
                    nc.vector.tensor_scalar(gt[i][:], gt[i][:],
                                            a_col[:, i:i + 1],
                                            c_col[:, i:i + 1],
                                            op0=OP.mult, op1=OP.add)
                else:
                    nc.scalar.activation(gt[i][:], gt[i][:], AF.Identity,
                                         bias=c_col[:, i:i + 1],
                                         scale=a_col[:, i:i + 1])
                dma_eng = nc.sync if i % 2 == 0 else nc.scalar
                dma_eng.dma_start(vout[b, 128 * ch:128 * (ch + 1), :],
                                  gt[i][:])

    nc.compile()
    return nc


def _get_nc():
    global _NC_CACHE
    if _NC_CACHE is None:
        _NC_CACHE = _build()
    return _NC_CACHE


def _make_host_inputs():
    iden = np.eye(D, dtype=np.float32)
    m16 = np.zeros((D, 16), dtype=np.float32)
    m16t = np.zeros((16, D), dtype=np.float32)
    for g in range(16):
        m16[8 * g:8 * g + 8, g] = 1.0
        m16t[g, 8 * g:8 * g + 8] = 1.0
    return iden, m16, m16t


def _make_sel(core):
    # T0' rows (AG-A): rank r block at 16r, local batches 0,1.
    # T1' rows (AG-B): rank r block at 16r, local batches 2,3.
    sel0 = np.zeros((D, NC32), dtype=np.float32)
    sel1 = np.zeros((D, NC32), dtype=np.float32)
    for b in range(B_LOC):
        for c in range(NCH):
            if b < 2:
                sel0[16 * core + 8 * b + c, 8 * b + c] = 1.0
            else:
                sel1[16 * core + 8 * (b - 2) + c, 8 * b + c] = 1.0
    return sel0, sel1


def _reference_numpy(x, bias):
    """General fallback (non-row-constant bias): straight numpy port."""
    x = x.astype(np.float32)
    bias = bias.astype(np.float32)
    u_norm = np.linalg.norm(x, axis=1)[..., None]
    u_hat = np.einsum('bdn,bdm->bnm', x, x)
    u_hat_norm = np.linalg.norm(u_hat, axis=-1, keepdims=True)
    new_norm = np.minimum(u_hat_norm, u_norm)
    u_hat = u_hat / u_hat_norm * new_norm
    t_num = np.float32(T_NUM)
    b_ij = np.zeros(u_hat.shape, dtype=np.float32)
    v_j = None
    for it in range(3):
        m = b_ij.max(axis=1, keepdims=True)
        e = np.exp(b_ij - m)
        c_ij = e / e.sum(axis=1, keepdims=True)
        s_j = c_ij * u_hat + bias
        sqn = np.sum(s_j * s_j, axis=-1, keepdims=True)
        v_j = sqn * s_j / ((1.0 + sqn) * np.sqrt(sqn))
        if it < 2:
            dd = np.linalg.norm(v_j - u_hat, axis=-1, keepdims=True)
            d_o = dd.mean()
            t = t_num / (0.5 * d_o - d_o + EPS)
            b_ij = t * dd
    return v_j


def kernel(x, bias):
    global LAST_EXEC_NS
    x = np.ascontiguousarray(x, dtype=np.float32)
    bias = np.ascontiguousarray(bias, dtype=np.float32)
    B = x.shape[0]
    row_const = bool((bias == bias[:, :, :1]).all())
    if not row_const or B != 32 or x.shape[1:] != (D, N):
        return _reference_numpy(x, bias)
    brow = np.ascontiguousarray(bias[0, :, 0]).astype(np.float32)  # (N,)
    iden, m16, m16t = _make_host_inputs()
    # bcol4[p, 8b+c] = bias[128c+p]; bb2[p', 128h+p] pattern rows repeat per 8
    bcol = brow.reshape(NCH, D).T  # (128, 8): [p, c]
    bcol4 = np.ascontiguousarray(np.tile(bcol, (1, B_LOC)))  # [p, 8b+c]? no:
    # tile gives [p, c*4] order [c0..c7,c0..c7,..]; need [8b+c] = same pattern
    # since tile repeats the 8-col block 4 times -> col index 8b+c maps to c ✓
    bb_row = (32.0 * brow) ** 2  # N*bias^2, (N,)
    bbp = bb_row.reshape(NCH, D)  # [c, p]
    bb128 = np.zeros((D, D), dtype=np.float32)  # [8g+c, p] rows repeat per 8
    for g in range(16):
        bb128[8 * g:8 * g + 8, :] = bbp
    bb2 = np.ascontiguousarray(np.concatenate([bb128, bb128], axis=1))
    nc = _get_nc()
    in_maps = []
    for core in range(N_CORES):
        sel0, sel1 = _make_sel(core)
        in_maps.append({
            "xs": np.ascontiguousarray(x[4 * core:4 * core + 4]),
            "iden": iden, "m16": m16, "m16t": m16t,
            "sel0": sel0, "sel1": sel1,
            "bcol4": bcol4, "bb2": bb2,
        })
    res = run_bass_kernel_spmd(nc, in_maps, core_ids=list(range(N_CORES)))
    LAST_EXEC_NS = res.exec_time_ns
    globals()["LAST_RES"] = res
    return np.concatenate([res.results[c]["v"] for c in range(N_CORES)], axis=0)
